# revision 14
# baseline (speedup 1.0000x reference)
"""Distributed GAT (2x GATConv + MLP self-path) on 8 Trainium2 NeuronCores.

Strategy (dst-node graph parallelism, SPMD on 8 cores, v3):
  Host:
    - fold attention vectors into projection weights, cast x/weights to bf16,
      append self-loops, sort edges by dst, partition edges by 6272-node
      (128-aligned) dst blocks per core, group each core's dst nodes into
      128-node groups, order each group's edges [src<LO | src>=LO] with both
      sections padded to chunk counts ch_lo/ch_hi (global constants so the
      SPMD program is identical across cores); pad gather indices with -1.
    - xp table columns are C-MAJOR per conv ([c*32+h]) so the per-edge
      per-head weighting broadcasts along the INNER head axis (DVE 2x mode).
    - stream the per-chunk transposed one-hot dst-selector S_T (bf16) so the
      per-edge a_d term becomes a tiny on-device matmul instead of a gather.
    - per-(group,call) gather num_idxs statically trimmed to the max real
      count over the 8 cores (desc-gen on the Q7 costs ~10-18ns per padded
      slot, so padding is expensive).
  Device:
    - Phase S: MLP self path (bf16 matmuls) on own node block.
    - Phase A: a_d for own dst nodes from x_own @ v_d (kept in SBUF).
    - Phase T: projection table row(n) = [xp1_cm 512|xp2_cm 512|a_s1 32|
      a_s2 32|pad 64] bf16 (2304 B) written as two tables (node < LO /
      >= LO so int16 gather indices reach every row).
    - Phase E: per 128-dst-node group: one dma_gather per section for all
      edges (both convs share the row), scores exp(leakyrelu(a_s+a_d)-ln16)
      without max-subtraction (exact softmax identity, scale cancels),
      one-hot segment matmuls accumulate the weighted feature sum and the
      softmax denominator in one PSUM tile; normalisation, head-mean (inner
      reduce in c-major), bias, elu.
  Host: concatenate per-core dst blocks.
"""
import math
import numpy as np
import ml_dtypes

import concourse.bass as bass
import concourse.tile as tile
import concourse.mybir as mybir
import bass_rust
from concourse import bacc
from concourse.bass_utils import run_bass_kernel_spmd

AX_X = bass_rust.AxisListType.X
F32 = mybir.dt.float32
BF16 = mybir.dt.bfloat16
I16 = mybir.dt.int16
Act = mybir.ActivationFunctionType
Alu = mybir.AluOpType
P = 128
LN16 = float(np.log(16.0))


class Cfg:
    def __init__(self):
        self.n = 50000
        self.d_in = 256
        self.c = 16
        self.h = 32
        self.hd = self.h * self.c           # 512
        self.n_cores = 8
        self.npc = 6272                     # 128-aligned dst block per core
        self.groups = self.npc // P         # 49
        self.n_pad = self.npc * self.n_cores  # 50176
        self.nt = self.n_pad // P           # 392 node tiles
        self.kd = self.d_in // P            # 2
        self.lo = 32768
        self.ntl = self.lo // P             # 256 tiles in lo table
        self.n_hi = self.n_pad - self.lo    # 17408
        self.rowe = 1152                    # table row bf16 elements (w/ pad)
        self.used = 1088                    # meaningful row columns
        self.sub = 4                        # chunks per span
        self.gcall = 8                      # chunks per gather call (<=64
                                            # descriptors per SDMA engine)


def _elu(nc, pool, out_ap, in_ap, tag):
    shape = list(in_ap.shape)
    u = pool.tile(shape, F32, tag=tag + "_u")
    rl = pool.tile(shape, F32, tag=tag + "_r")
    nc.vector.tensor_scalar_min(out=u[:], in0=in_ap, scalar1=0.0)
    nc.scalar.activation(u[:], u[:], Act.Exp)
    nc.scalar.activation(rl[:], in_ap, Act.Relu)
    nc.vector.scalar_tensor_tensor(
        out=out_ap, in0=u[:], scalar=-1.0, in1=rl[:], op0=Alu.add, op1=Alu.add)


def build_program(cfg: Cfg, ch_lo: int, ch_hi: int, nidx, phases: str = "STE"):
    """nidx: per-group tuple of per-call static gather num_idxs (128-aligned,
    maxed over cores, <=1024 each; 0 = skip the call). Call windows are
    gcall-chunk slices of the lo then hi sections."""
    nc = bacc.Bacc("TRN2", target_bir_lowering=False, debug=False,
                   num_devices=cfg.n_cores)
    G, H, C, HD = cfg.groups, cfg.h, cfg.c, cfg.hd
    ch = ch_lo + ch_hi
    ROWE = cfg.rowe
    SUB = cfg.sub
    GC = cfg.gcall
    calls_lo = math.ceil(ch_lo / GC)
    calls_hi = math.ceil(ch_hi / GC)
    ncalls = calls_lo + calls_hi
    # (chunk base, table) per call window
    windows = [(ci * GC, 0) for ci in range(calls_lo)] + \
              [(ch_lo + ci * GC, 1) for ci in range(calls_hi)]

    t_xT = nc.dram_tensor("xT", [cfg.d_in, cfg.n_pad], BF16, kind="ExternalInput")
    t_xTs = nc.dram_tensor("xTs", [cfg.d_in, cfg.npc], BF16, kind="ExternalInput")
    t_wcat = nc.dram_tensor("wcat", [cfg.d_in, cfg.used], BF16, kind="ExternalInput")
    t_vd = nc.dram_tensor("vd", [cfg.d_in, 2 * H], BF16, kind="ExternalInput")
    t_l1w = nc.dram_tensor("l1w", [cfg.d_in, 4 * C], BF16, kind="ExternalInput")
    t_l2w = nc.dram_tensor("l2w", [4 * C, C], BF16, kind="ExternalInput")
    t_l1b = nc.dram_tensor("l1b", [4 * C, 1], F32, kind="ExternalInput")
    t_l2b = nc.dram_tensor("l2b", [P, C], F32, kind="ExternalInput")
    t_bcat = nc.dram_tensor("bcat", [P, 2 * C], F32, kind="ExternalInput")
    t_esrc = nc.dram_tensor("esrc", [P, G * ch * 8], I16, kind="ExternalInput")
    t_cnt = nc.dram_tensor("cnt", [P, G * ncalls], mybir.dt.int32,
                           kind="ExternalInput")
    t_erel = nc.dram_tensor("erel", [P, G * ch], BF16, kind="ExternalInput")
    t_sT = nc.dram_tensor("sT", [P, G * ch * P], BF16, kind="ExternalInput")

    t_oconv = [nc.dram_tensor(f"out_conv{i}", [cfg.npc, C], F32,
                              kind="ExternalOutput") for i in range(2)]
    t_oself = nc.dram_tensor("out_self", [cfg.npc, C], F32,
                             kind="ExternalOutput")

    t_tabL = nc.dram_tensor("tabL", [cfg.lo, ROWE], BF16)
    t_tabH = nc.dram_tensor("tabH", [cfg.n_hi, ROWE], BF16)
    # per-group hi-section partial sums (spilled in window A, folded in B)
    t_part = nc.dram_tensor("part", [G * P, cfg.used], BF16)

    with tile.TileContext(nc) as tc:
        import contextlib
        with contextlib.ExitStack() as ctx:
            cst = ctx.enter_context(tc.tile_pool(name="cst", bufs=1))
            wkp = ctx.enter_context(tc.tile_pool(name="wkp", bufs=2))
            stc = contextlib.ExitStack()
            cstT = stc.enter_context(tc.tile_pool(name="cstT", bufs=1))
            xtp = stc.enter_context(tc.tile_pool(name="xtp", bufs=2))
            tbp = stc.enter_context(tc.tile_pool(name="tbp", bufs=3))

            # ---- constants ----
            iota_i = cst.tile([P, P], mybir.dt.int32, tag="ioi")
            nc.gpsimd.iota(iota_i[:], pattern=[[1, P]], base=0, channel_multiplier=0)
            iota_b = cst.tile([P, P], BF16, tag="iob")
            nc.vector.tensor_copy(iota_b[:], iota_i[:])
            iota_c = cst.tile([P, P], mybir.dt.int32, tag="ioc")
            nc.gpsimd.iota(iota_c[:], pattern=[[0, P]], base=0, channel_multiplier=1)
            ident = cst.tile([P, P], BF16, tag="ident")
            nc.vector.tensor_tensor(out=ident[:], in0=iota_c[:], in1=iota_i[:],
                                    op=Alu.is_equal)
            wcat = []
            for k in range(cfg.kd):
                w = cstT.tile([P, cfg.used], BF16, tag=f"wc{k}")
                nc.sync.dma_start(out=w[:], in_=t_wcat[k * P:(k + 1) * P, :])
                wcat.append(w)
            vd = []
            for k in range(cfg.kd):
                w = cstT.tile([P, 2 * H], BF16, tag=f"vd{k}")
                nc.sync.dma_start(out=w[:], in_=t_vd[k * P:(k + 1) * P, :])
                vd.append(w)
            l1w = []
            for k in range(cfg.kd):
                w = cstT.tile([P, 4 * C], BF16, tag=f"l1w{k}")
                nc.sync.dma_start(out=w[:], in_=t_l1w[k * P:(k + 1) * P, :])
                l1w.append(w)
            l2w = cstT.tile([4 * C, C], BF16, tag="l2w")
            nc.sync.dma_start(out=l2w[:], in_=t_l2w[:])
            l1b = cstT.tile([4 * C, 1], F32, tag="l1b")
            nc.sync.dma_start(out=l1b[:], in_=t_l1b[:])
            l2b = cstT.tile([P, C], F32, tag="l2b")
            nc.sync.dma_start(out=l2b[:], in_=t_l2b[:])
            bcat = cst.tile([P, 2, C], F32, tag="bcat")
            nc.sync.dma_start(out=bcat[:].rearrange("p u c -> p (u c)"),
                              in_=t_bcat[:])
            nl16 = cst.tile([P, 1], F32, tag="nl16")
            nc.gpsimd.memset(nl16[:], -LN16)
            erel = cst.tile([P, G * ch], BF16, tag="erel")
            nc.sync.dma_start(out=erel[:], in_=t_erel[:])
            cnt = cst.tile([P, G * ncalls], mybir.dt.int32, tag="cnt")
            nc.sync.dma_start(out=cnt[:], in_=t_cnt[:])
            adn = cst.tile([P, G, 2 * H], BF16, tag="adn")

            # ---- Phase S: self path + Phase A: own-node a_d ----
            with tc.tile_pool(name="psS", bufs=2, space="PSUM") as psS, \
                 tc.tile_pool(name="psT", bufs=2, space="PSUM") as psT:
                blk = 0
                while "S" in phases and blk < cfg.npc:
                    bs = min(512, cfg.npc - blk)
                    x1p = psS.tile([4 * C, 512], F32, tag="x1p")
                    xk = []
                    for k in range(cfg.kd):
                        xts = xtp.tile([P, 512], BF16, tag="xts")
                        nc.sync.dma_start(out=xts[:, :bs],
                                          in_=t_xTs[k * P:(k + 1) * P, blk:blk + bs])
                        xk.append(xts)
                        nc.tensor.matmul(out=x1p[:, :bs], lhsT=l1w[k][:],
                                         rhs=xts[:, :bs],
                                         start=(k == 0), stop=(k == cfg.kd - 1))
                    x1s = wkp.tile([4 * C, 512], BF16, tag="x1s")
                    nc.vector.tensor_add(out=x1s[:, :bs], in0=x1p[:, :bs],
                                         in1=l1b[:].to_broadcast([4 * C, bs]))
                    _elu(nc, wkp, x1s[:, :bs], x1s[:, :bs], "se")
                    for m in range(bs // P):
                        gi = (blk + m * P) // P
                        o2p = psT.tile([P, C], F32, tag="o2p")
                        nc.tensor.matmul(out=o2p[:],
                                         lhsT=x1s[:, m * P:(m + 1) * P],
                                         rhs=l2w[:], start=True, stop=True)
                        o2s = wkp.tile([P, C], F32, tag="o2s")
                        nc.vector.tensor_add(out=o2s[:], in0=o2p[:], in1=l2b[:])
                        _elu(nc, wkp, o2s[:], o2s[:], "so")
                        nc.sync.dma_start(
                            out=t_oself[blk + m * P:blk + (m + 1) * P, :],
                            in_=o2s[:])
                        # own-node a_d for this 128-node group
                        adp = psT.tile([P, 2 * H], F32, tag="adp")
                        for k in range(cfg.kd):
                            nc.tensor.matmul(out=adp[:],
                                             lhsT=xk[k][:, m * P:(m + 1) * P],
                                             rhs=vd[k][:],
                                             start=(k == 0), stop=(k == cfg.kd - 1))
                        nc.vector.tensor_copy(out=adn[:, gi, :], in_=adp[:])
                    blk += bs

            QS = ((0, 512), (512, 1024), (1024, 1088))

            def emit_T_batch(ts4):
                mt = min(4, cfg.nt - ts4)
                xk = []
                for k in range(cfg.kd):
                    xt = xtp.tile([P, 512], BF16, tag=f"xt{k}")
                    nc.sync.dma_start(
                        out=xt[:, :mt * P],
                        in_=t_xT[k * P:(k + 1) * P, ts4 * P:(ts4 + mt) * P])
                    xk.append(xt)
                for m in range(mt):
                    ts = ts4 + m
                    pt = psA.tile([P, cfg.used], F32, tag="pt")
                    for k in range(cfg.kd):
                        for q0, q1 in QS:
                            nc.tensor.matmul(out=pt[:, q0:q1],
                                             lhsT=xk[k][:, m * P:(m + 1) * P],
                                             rhs=wcat[k][:, q0:q1],
                                             start=(k == 0),
                                             stop=(k == cfg.kd - 1))
                    # table row: [xp1_cm 512 | xp2_cm 512 | a_s1 32 |
                    #             a_s2 32 | pad 64]
                    stag = tbp.tile([P, ROWE], BF16, tag="stag")
                    nc.vector.tensor_copy(
                        out=stag[:, 0:256], in_=pt[:, 0:256])
                    nc.scalar.activation(
                        stag[:, 256:1088], pt[:, 256:1088], Act.Copy)
                    if ts < cfg.ntl:
                        rows = t_tabL[ts * P:(ts + 1) * P, :]
                    else:
                        rows = t_tabH[(ts - cfg.ntl) * P:(ts - cfg.ntl + 1) * P, :]
                    nc.sync.dma_start(out=rows, in_=stag[:])

            def emit_spans(gi, xpa, sTl, out_ps, nch, gcb, first, last):
                """Edge spans for nch chunks of group gi.

                xpa: [P, nch, ROWE] gathered rows; sTl: [P, nch*P] dst one-hot
                (transposed); gcb: global chunk base (erel/sT column offset is
                handled by caller for sT; erel uses gcb). first/last: this
                call opens/closes the PSUM accumulation group."""
                pair = 0
                nch_all = nch
                for c0 in range(0, nch_all, SUB):
                    jw = min(SUB, nch_all - c0)
                    ade = psD.tile([P, SUB, 2 * H], F32, tag="ade")
                    for j in range(jw):
                        nc.tensor.matmul(
                            out=ade[:, j, :],
                            lhsT=sTl[:, (c0 + j) * P:(c0 + j + 1) * P],
                            rhs=adn[:, gi, :], start=True, stop=True)
                    scr = wsp.tile([P, SUB, 2 * H], BF16, tag="scr")
                    nc.vector.tensor_add(
                        out=scr[:, :jw, :].rearrange(
                            "p s (u t) -> p s u t", u=2),
                        in0=xpa[:, c0:c0 + jw, 1024:1088].rearrange(
                            "p s (u t) -> p s u t", u=2),
                        in1=ade[:, :jw, :].rearrange(
                            "p s (u t) -> p s u t", u=2))
                    nc.vector.scalar_tensor_tensor(
                        out=scr[:, :jw, :], in0=scr[:, :jw, :], scalar=0.2,
                        in1=scr[:, :jw, :], op0=Alu.mult, op1=Alu.max)
                    wsc = wsp.tile([P, SUB, cfg.used], BF16, tag="wsc")
                    nc.scalar.activation(
                        wsc[:, :jw, 1024:1088], scr[:, :jw, :], Act.Exp,
                        bias=nl16[:], scale=1.0)
                    for i in range(2):
                        nc.vector.tensor_tensor(
                            out=wsc[:, :jw, i * HD:(i + 1) * HD].rearrange(
                                "p s (c h) -> p s c h", c=C),
                            in0=xpa[:, c0:c0 + jw,
                                    i * HD:(i + 1) * HD].rearrange(
                                "p s (c h) -> p s c h", c=C),
                            in1=wsc[:, :jw, 1024 + i * H:1024 + (i + 1) * H]
                                .unsqueeze(2).to_broadcast([P, jw, C, H]),
                            op=Alu.mult)
                    S = wsp.tile([P, SUB, P], BF16, tag="S")
                    nc.vector.tensor_tensor(
                        out=S[:, :jw, :],
                        in0=erel[:, gcb + c0:gcb + c0 + jw]
                            .unsqueeze(2).to_broadcast([P, jw, P]),
                        in1=iota_b[:].unsqueeze(1).to_broadcast([P, jw, P]),
                        op=Alu.is_equal)
                    for jj in range(jw):
                        st = first and pair == 0
                        sp = last and pair == nch_all - 1
                        for q0, q1 in QS:
                            nc.tensor.matmul(
                                out=out_ps[:, q0:q1],
                                lhsT=S[:, jj, :],
                                rhs=wsc[:, jj, q0:q1],
                                start=st, stop=sp)
                        pair += 1

            def emit_gather(gi, xpa, isrc, wins, greg):
                for c0loc, ci, sec in wins:
                    nd = nidx[gi][ci]
                    if nd == 0:
                        continue
                    tab = t_tabL if sec == 0 else t_tabH
                    jw = (nd + P - 1) // P
                    nc.gpsimd.reg_load(
                        greg, cnt[0:1, gi * ncalls + ci:gi * ncalls + ci + 1])
                    nc.gpsimd.dma_gather(
                        out_ap=xpa[:, c0loc:c0loc + jw, :], in_ap=tab[:],
                        idxs_ap=isrc[:, c0loc * 8:c0loc * 8 + nd // 16],
                        num_idxs=nd, num_idxs_reg=greg,
                        elem_size=ROWE)

            # ---- Phase T-hi: high table (nodes >= LO) ----
            with tc.tile_pool(name="psA", bufs=1, space="PSUM") as psA:
                for ts4 in (range(cfg.ntl, cfg.nt, 4) if "T" in phases else []):
                    emit_T_batch(ts4)

                # ---- Window A: T-lo interleaved with E-hi (partials) ----
                hi_wins = [(c0 - ch_lo, ci, sec)
                           for ci, (c0, sec) in enumerate(windows) if sec == 1]
                XBH = 2
                with tc.tile_pool(name="gath", bufs=XBH) as gath, \
                     tc.tile_pool(name="stph", bufs=2) as stph, \
                     tc.tile_pool(name="wsp", bufs=2) as wsp, \
                     tc.tile_pool(name="spp", bufs=2) as spp, \
                     tc.tile_pool(name="psE", bufs=1, space="PSUM") as psE, \
                     tc.tile_pool(name="psD", bufs=2, space="PSUM") as psD, \
                     nc.gpsimd.register("gcnth") as gregh:
                    lo_batches = list(range(0, cfg.ntl, 4)) if "T" in phases else []
                    bi = 0
                    for gi in (range(G) if "E" in phases else []):
                        tgt = (gi + 1) * len(lo_batches) // G
                        while bi < tgt:
                            emit_T_batch(lo_batches[bi])
                            bi += 1
                        isrch = gath.tile([P, ch_hi * 8], I16, tag="isrch")
                        nc.sync.dma_start(
                            out=isrch[:],
                            in_=t_esrc[:, (gi * ch + ch_lo) * 8:
                                       (gi * ch + ch) * 8])
                        sTh = stph.tile([P, ch_hi * P], BF16, tag="sTh")
                        nc.sync.dma_start(
                            out=sTh[:],
                            in_=t_sT[:, (gi * ch + ch_lo) * P:
                                     (gi * ch + ch) * P])
                        xpah = gath.tile([P, ch_hi, ROWE], BF16, tag="xpah")
                        if gi < XBH:
                            nc.gpsimd.memset(xpah[:], 0.0)
                        emit_gather(gi, xpah, isrch, hi_wins, gregh)
                        out_ps = psE.tile([P, cfg.used], F32, tag="out_ps")
                        emit_spans(gi, xpah, sTh, out_ps, ch_hi,
                                   gi * ch + ch_lo, first=True, last=True)
                        sp = spp.tile([P, cfg.used], BF16, tag="sp")
                        nc.scalar.activation(sp[:], out_ps[:], Act.Copy)
                        nc.sync.dma_start(
                            out=t_part[gi * P:(gi + 1) * P, :], in_=sp[:])
                    while bi < len(lo_batches):
                        emit_T_batch(lo_batches[bi])
                        bi += 1

            stc.close()

            # ---- Window B: E-lo + partial fold + finalize ----
            XB = 3
            lo_wins = [(c0, ci, sec)
                       for ci, (c0, sec) in enumerate(windows) if sec == 0]
            with tc.tile_pool(name="gat", bufs=XB) as gat, \
                 tc.tile_pool(name="stp", bufs=2) as stp, \
                 tc.tile_pool(name="wspB", bufs=2) as wsp, \
                 tc.tile_pool(name="ptp", bufs=2) as ptp, \
                 tc.tile_pool(name="psEB", bufs=2, space="PSUM") as psE, \
                 tc.tile_pool(name="psDB", bufs=2, space="PSUM") as psD, \
                 nc.gpsimd.register("gcnt") as greg:
                for gi in (range(G) if "E" in phases else []):
                    isrc = gat.tile([P, ch_lo * 8], I16, tag="isrc")
                    nc.sync.dma_start(
                        out=isrc[:],
                        in_=t_esrc[:, gi * ch * 8:(gi * ch + ch_lo) * 8])
                    sTl = stp.tile([P, ch_lo * P], BF16, tag="sTl")
                    nc.sync.dma_start(
                        out=sTl[:],
                        in_=t_sT[:, gi * ch * P:(gi * ch + ch_lo) * P])
                    part = ptp.tile([P, cfg.used], BF16, tag="part")
                    nc.sync.dma_start(out=part[:],
                                      in_=t_part[gi * P:(gi + 1) * P, :])
                    xpa = gat.tile([P, ch_lo, ROWE], BF16, tag="xpa")
                    if gi < XB:
                        nc.vector.memset(xpa[:], 0.0)
                    emit_gather(gi, xpa, isrc, lo_wins, greg)
                    out_ps = psE.tile([P, cfg.used], F32, tag="out_ps")
                    emit_spans(gi, xpa, sTl, out_ps, ch_lo, gi * ch,
                               first=True, last=False)
                    # fold the hi partial back in (identity matmul accumulate)
                    for qi, (q0, q1) in enumerate(QS):
                        nc.tensor.matmul(out=out_ps[:, q0:q1], lhsT=ident[:],
                                         rhs=part[:, q0:q1],
                                         start=False, stop=True)
                    # normalize + head mean (inner) + bias + elu
                    rsb = wkp.tile([P, 2 * H], F32, tag="rsb")
                    nc.scalar.activation(rsb[:], out_ps[:, 1024:1088],
                                         Act.Copy, scale=float(H), bias=1e-12)
                    nc.vector.reciprocal(rsb[:], rsb[:])
                    tmp = wkp.tile([P, 2, C, H], F32, tag="tmp")
                    nc.vector.tensor_tensor(
                        out=tmp[:],
                        in0=out_ps[:, 0:1024].rearrange(
                            "p (u c h) -> p u c h", u=2, c=C),
                        in1=rsb[:].rearrange("p (u h) -> p u h", u=2)
                            .unsqueeze(2).to_broadcast([P, 2, C, H]),
                        op=Alu.mult)
                    om = wkp.tile([P, 2, C], F32, tag="om")
                    nc.vector.reduce_sum(out=om[:], in_=tmp[:], axis=AX_X)
                    nc.vector.tensor_add(out=om[:], in0=om[:], in1=bcat[:])
                    _elu(nc, wkp, om[:], om[:], "oe")
                    for i in range(2):
                        nc.sync.dma_start(
                            out=t_oconv[i][gi * P:(gi + 1) * P, :],
                            in_=om[:, i, :])
    nc.compile()
    return nc


def _wrap16(flat):
    """edge i -> [i%16, i//16], replicated to 128 partitions."""
    w = flat.reshape(-1, 16).T  # [16, len/16]
    return np.tile(w, (8, 1))


def preprocess(cfg: Cfg, inputs):
    n, H, C, G = cfg.n, cfg.h, cfg.c, cfg.groups
    x = np.asarray(inputs["x"], np.float32)
    ei = np.asarray(inputs["edge_index"])

    def fold(W, a_s, a_d):
        W = np.asarray(W, np.float32).reshape(cfg.d_in, H, C)
        v_s = (W * np.asarray(a_s, np.float32)[None]).sum(-1)
        v_d = (W * np.asarray(a_d, np.float32)[None]).sum(-1)
        # c-major projection columns: col (c*H + h) = W[:, h, c]
        W_cm = W.transpose(0, 2, 1).reshape(cfg.d_in, H * C)
        return W_cm, v_s, v_d

    W1, vs1, vd1 = fold(inputs["W1"], inputs["att_src1"], inputs["att_dst1"])
    W2, vs2, vd2 = fold(inputs["W2"], inputs["att_src2"], inputs["att_dst2"])
    # psum col layout: [xp1_cm 512 | xp2_cm 512 | a_s1 32 | a_s2 32]
    wcat = np.concatenate([W1, W2, vs1, vs2], 1).astype(ml_dtypes.bfloat16)
    vdcat = np.concatenate([vd1, vd2], 1).astype(ml_dtypes.bfloat16)

    xT = np.zeros((cfg.d_in, cfg.n_pad), ml_dtypes.bfloat16)
    xT[:, :n] = x.T.astype(ml_dtypes.bfloat16)

    loops = np.arange(n, dtype=np.int64)
    src = np.concatenate([ei[0], loops]).astype(np.int32)
    dst = np.concatenate([ei[1], loops]).astype(np.int32)
    order = np.argsort(dst, kind="stable")
    src_s = src[order]
    dst_s = dst[order]
    dev = dst_s // cfg.npc
    rel = dst_s - dev * cfg.npc
    grp = rel >> 7
    hi_e = (src_s >= cfg.lo).astype(np.int64)
    sidx = (dev * G + grp) * 2 + hi_e
    counts = np.bincount(sidx, minlength=cfg.n_cores * G * 2)
    c2 = counts.reshape(-1, 2)

    ch_lo = max(1, int(math.ceil(c2[:, 0].max() / P)))
    ch_hi = max(1, int(math.ceil(c2[:, 1].max() / P)))
    ch = ch_lo + ch_hi
    order2 = np.argsort(sidx, kind="stable")
    src2 = src_s[order2]
    sidx2 = sidx[order2]
    rel2 = (rel & 127)[order2]
    starts = np.zeros(len(counts), np.int64)
    starts[1:] = np.cumsum(counts)[:-1]
    pos_in_sec = np.arange(len(src2)) - starts[sidx2]
    dev2 = sidx2 // (G * 2)
    grp2 = (sidx2 // 2) % G
    hi2 = sidx2 & 1
    slot = np.where(hi2 == 0, pos_in_sec, ch_lo * P + pos_in_sec)
    gpos = grp2 * (ch * P) + slot

    GE = G * ch * P
    src16 = np.full((cfg.n_cores, GE), -1, np.int16)
    relf = np.full((cfg.n_cores, GE), -1.0, np.float32)
    sTf = np.zeros((cfg.n_cores, P, GE), ml_dtypes.bfloat16)
    src_adj = np.where(hi2 == 1, src2 - cfg.lo, src2).astype(np.int16)
    src16[dev2, gpos] = src_adj
    relf[dev2, gpos] = rel2.astype(np.float32)
    sTf[dev2, rel2, gpos] = 1.0

    # per-(core, group, section) real counts
    secn = np.zeros((cfg.n_cores, G, 2), np.int64)
    np.add.at(secn, (dev2, grp2, hi2), 1)
    # gather call windows: gcall-chunk slices of each section
    GC = cfg.gcall
    calls_lo = math.ceil(ch_lo / GC)
    calls_hi = math.ceil(ch_hi / GC)
    ncalls = calls_lo + calls_hi
    cnts = np.zeros((cfg.n_cores, G, ncalls), np.int32)
    nidx = []
    nmax = np.maximum(secn.max(axis=0), 1)          # [G, 2]
    for ci in range(ncalls):
        if ci < calls_lo:
            s, base = 0, ci * GC * P
            width = min(GC, ch_lo - ci * GC) * P
        else:
            s, base = 1, (ci - calls_lo) * GC * P
            width = min(GC, ch_hi - (ci - calls_lo) * GC) * P
        real = np.clip(secn[:, :, s] - base, 0, width)        # [cores, G]
        stat = np.clip(-(-(nmax[:, s] - base) // P) * P, 0, width)  # [G]
        cnts[:, :, ci] = np.where(real > 0, real, 1)
        nidx.append(stat)
        # sentinel index 0 for cores with an empty (but emitted) window
        need = (real == 0) & (stat[None, :] > 0)
        if need.any():
            cc, gg = np.nonzero(need)
            sec_off = 0 if s == 0 else ch_lo * P
            src16[cc, gg * (ch * P) + sec_off + base] = 0
    nidx = tuple(tuple(int(nidx[ci][g]) for ci in range(ncalls))
                 for g in range(G))

    l1b = np.asarray(inputs["lin1_b"], np.float32).reshape(4 * C, 1)
    l2b = np.broadcast_to(np.asarray(inputs["lin2_b"], np.float32), (P, C)).copy()
    bcat = np.broadcast_to(
        np.concatenate([np.asarray(inputs["b1"], np.float32),
                        np.asarray(inputs["b2"], np.float32)]), (P, 2 * C)).copy()

    in_maps = []
    for c in range(cfg.n_cores):
        xTs = np.zeros((cfg.d_in, cfg.npc), ml_dtypes.bfloat16)
        lo_n = min(cfg.npc, max(0, n - c * cfg.npc))
        xTs[:, :lo_n] = xT[:, c * cfg.npc:c * cfg.npc + lo_n]
        esrc = np.concatenate(
            [_wrap16(src16[c, gi * ch * P:(gi + 1) * ch * P])
             for gi in range(G)], axis=1)
        erel = np.concatenate(
            [relf[c, gi * ch * P:(gi + 1) * ch * P].reshape(ch, P).T
             for gi in range(G)], axis=1).astype(ml_dtypes.bfloat16)
        cntc = np.broadcast_to(cnts[c].reshape(1, -1),
                               (P, cnts.shape[1] * cnts.shape[2])).copy()
        in_maps.append({
            "xT": xT, "xTs": xTs, "wcat": wcat, "vd": vdcat, "cnt": cntc,
            "l1w": np.asarray(inputs["lin1_w"], np.float32).astype(
                ml_dtypes.bfloat16),
            "l2w": np.asarray(inputs["lin2_w"], np.float32).astype(
                ml_dtypes.bfloat16),
            "l1b": l1b, "l2b": l2b, "bcat": bcat,
            "esrc": np.ascontiguousarray(esrc),
            "erel": np.ascontiguousarray(erel),
            "sT": np.ascontiguousarray(sTf[c]),
        })
    return in_maps, ch_lo, ch_hi, nidx


_CACHE = {}


def kernel(**inputs):
    cfg = Cfg()
    in_maps, ch_lo, ch_hi, nidx = preprocess(cfg, inputs)
    key = ("v4", ch_lo, ch_hi, nidx)
    if key not in _CACHE:
        _CACHE[key] = build_program(cfg, ch_lo, ch_hi, nidx)
    nc = _CACHE[key]
    res = run_bass_kernel_spmd(nc, in_maps, list(range(cfg.n_cores))).results
    takes = [min(cfg.npc, cfg.n - c * cfg.npc) for c in range(cfg.n_cores)]
    x_in = np.concatenate([res[c]["out_conv0"][:takes[c]]
                           for c in range(cfg.n_cores)])
    x_out = np.concatenate([res[c]["out_conv1"][:takes[c]]
                            for c in range(cfg.n_cores)])
    x_self = np.concatenate([res[c]["out_self"][:takes[c]]
                             for c in range(cfg.n_cores)])
    return (x_in, x_out, x_self)


# revision 15
# speedup vs baseline: 1.4341x; 1.4341x over previous
"""Distributed GAT (2x GATConv + MLP self-path) on 8 Trainium2 NeuronCores.

Strategy (dst-node graph parallelism, SPMD on 8 cores, v3):
  Host:
    - fold attention vectors into projection weights, cast x/weights to bf16,
      append self-loops, sort edges by dst, partition edges by 6272-node
      (128-aligned) dst blocks per core, group each core's dst nodes into
      128-node groups, order each group's edges [src<LO | src>=LO] with both
      sections padded to chunk counts ch_lo/ch_hi (global constants so the
      SPMD program is identical across cores); pad gather indices with -1.
    - xp table columns are C-MAJOR per conv ([c*32+h]) so the per-edge
      per-head weighting broadcasts along the INNER head axis (DVE 2x mode).
    - stream the per-chunk transposed one-hot dst-selector S_T (bf16) so the
      per-edge a_d term becomes a tiny on-device matmul instead of a gather.
    - per-(group,call) gather num_idxs statically trimmed to the max real
      count over the 8 cores (desc-gen on the Q7 costs ~10-18ns per padded
      slot, so padding is expensive).
  Device:
    - Phase S: MLP self path (bf16 matmuls) on own node block.
    - Phase A: a_d for own dst nodes from x_own @ v_d (kept in SBUF).
    - Phase T: projection table row(n) = [xp1_cm 512|xp2_cm 512|a_s1 32|
      a_s2 32|pad 64] bf16 (2304 B) written as two tables (node < LO /
      >= LO so int16 gather indices reach every row).
    - Phase E: per 128-dst-node group: one dma_gather per section for all
      edges (both convs share the row), scores exp(leakyrelu(a_s+a_d)-ln16)
      without max-subtraction (exact softmax identity, scale cancels),
      one-hot segment matmuls accumulate the weighted feature sum and the
      softmax denominator in one PSUM tile; normalisation, head-mean (inner
      reduce in c-major), bias, elu.
  Host: concatenate per-core dst blocks.
"""
import math
import numpy as np
import ml_dtypes

import concourse.bass as bass
import concourse.tile as tile
import concourse.mybir as mybir
import bass_rust
from concourse import bacc
from concourse.bass_utils import run_bass_kernel_spmd

AX_X = bass_rust.AxisListType.X
F32 = mybir.dt.float32
BF16 = mybir.dt.bfloat16
I16 = mybir.dt.int16
Act = mybir.ActivationFunctionType
Alu = mybir.AluOpType
P = 128
LN16 = float(np.log(16.0))


class Cfg:
    def __init__(self):
        self.n = 50000
        self.d_in = 256
        self.c = 16
        self.h = 32
        self.hd = self.h * self.c           # 512
        self.n_cores = 8
        self.npc = 6272                     # 128-aligned dst block per core
        self.groups = self.npc // P         # 49
        self.n_pad = self.npc * self.n_cores  # 50176
        self.nt = self.n_pad // P           # 392 node tiles
        self.kd = self.d_in // P            # 2
        self.lo = 32768
        self.ntl = self.lo // P             # 256 tiles in lo table
        self.n_hi = self.n_pad - self.lo    # 17408
        self.rowe = 1152                    # table row bf16 elements (w/ pad)
        self.used = 1088                    # meaningful row columns
        self.sub = 4                        # chunks per span
        self.gcall = 8                      # chunks per gather call (<=64
                                            # descriptors per SDMA engine)


def _elu(nc, pool, out_ap, in_ap, tag):
    shape = list(in_ap.shape)
    u = pool.tile(shape, F32, tag=tag + "_u")
    rl = pool.tile(shape, F32, tag=tag + "_r")
    nc.vector.tensor_scalar_min(out=u[:], in0=in_ap, scalar1=0.0)
    nc.scalar.activation(u[:], u[:], Act.Exp)
    nc.scalar.activation(rl[:], in_ap, Act.Relu)
    nc.vector.scalar_tensor_tensor(
        out=out_ap, in0=u[:], scalar=-1.0, in1=rl[:], op0=Alu.add, op1=Alu.add)


def build_program(cfg: Cfg, ch_lo: int, ch_hi: int, nidx, phases: str = "STE"):
    """nidx: per-group tuple of per-call static gather num_idxs (128-aligned,
    maxed over cores, <=1024 each; 0 = skip the call). Call windows are
    gcall-chunk slices of the lo then hi sections."""
    nc = bacc.Bacc("TRN2", target_bir_lowering=False, debug=False,
                   num_devices=cfg.n_cores)
    G, H, C, HD = cfg.groups, cfg.h, cfg.c, cfg.hd
    ch = ch_lo + ch_hi
    ROWE = cfg.rowe
    SUB = cfg.sub
    GC = cfg.gcall
    calls_lo = math.ceil(ch_lo / GC)
    calls_hi = math.ceil(ch_hi / GC)
    ncalls = calls_lo + calls_hi
    # (chunk base, table) per call window
    windows = [(ci * GC, 0) for ci in range(calls_lo)] + \
              [(ch_lo + ci * GC, 1) for ci in range(calls_hi)]

    t_xT = nc.dram_tensor("xT", [cfg.d_in, cfg.n_pad], BF16, kind="ExternalInput")
    t_xTs = nc.dram_tensor("xTs", [cfg.d_in, cfg.npc], BF16, kind="ExternalInput")
    t_wcat = nc.dram_tensor("wcat", [cfg.d_in, cfg.used], BF16, kind="ExternalInput")
    t_vd = nc.dram_tensor("vd", [cfg.d_in, 2 * H], BF16, kind="ExternalInput")
    t_l1w = nc.dram_tensor("l1w", [cfg.d_in, 4 * C], BF16, kind="ExternalInput")
    t_l2w = nc.dram_tensor("l2w", [4 * C, C], BF16, kind="ExternalInput")
    t_l1b = nc.dram_tensor("l1b", [4 * C, 1], F32, kind="ExternalInput")
    t_l2b = nc.dram_tensor("l2b", [P, C], F32, kind="ExternalInput")
    t_bcat = nc.dram_tensor("bcat", [P, 2 * C], F32, kind="ExternalInput")
    t_esrc = nc.dram_tensor("esrc", [P, G * ch * 8], I16, kind="ExternalInput")
    t_cnt = nc.dram_tensor("cnt", [P, G * ncalls], mybir.dt.int32,
                           kind="ExternalInput")
    t_erel = nc.dram_tensor("erel", [P, G * ch], BF16, kind="ExternalInput")
    t_sT = nc.dram_tensor("sT", [P, G * ch * P], BF16, kind="ExternalInput")

    t_oconv = [nc.dram_tensor(f"out_conv{i}", [cfg.npc, C], F32,
                              kind="ExternalOutput") for i in range(2)]
    t_oself = nc.dram_tensor("out_self", [cfg.npc, C], F32,
                             kind="ExternalOutput")

    t_tabL = nc.dram_tensor("tabL", [cfg.lo, ROWE], BF16)
    t_tabH = nc.dram_tensor("tabH", [cfg.n_hi, ROWE], BF16)

    with tile.TileContext(nc) as tc:
        import contextlib
        with contextlib.ExitStack() as ctx:
            cst = ctx.enter_context(tc.tile_pool(name="cst", bufs=1))
            wkp = ctx.enter_context(tc.tile_pool(name="wkp", bufs=2))
            stc = contextlib.ExitStack()
            cstT = stc.enter_context(tc.tile_pool(name="cstT", bufs=1))
            xtp = stc.enter_context(tc.tile_pool(name="xtp", bufs=2))
            tbp = stc.enter_context(tc.tile_pool(name="tbp", bufs=3))

            # ---- constants ----
            iota_i = cst.tile([P, P], mybir.dt.int32, tag="ioi")
            nc.gpsimd.iota(iota_i[:], pattern=[[1, P]], base=0, channel_multiplier=0)
            iota_b = cst.tile([P, P], BF16, tag="iob")
            nc.vector.tensor_copy(iota_b[:], iota_i[:])
            wcat = []
            for k in range(cfg.kd):
                w = cstT.tile([P, cfg.used], BF16, tag=f"wc{k}")
                nc.sync.dma_start(out=w[:], in_=t_wcat[k * P:(k + 1) * P, :])
                wcat.append(w)
            vd = []
            for k in range(cfg.kd):
                w = cstT.tile([P, 2 * H], BF16, tag=f"vd{k}")
                nc.sync.dma_start(out=w[:], in_=t_vd[k * P:(k + 1) * P, :])
                vd.append(w)
            l1w = []
            for k in range(cfg.kd):
                w = cstT.tile([P, 4 * C], BF16, tag=f"l1w{k}")
                nc.sync.dma_start(out=w[:], in_=t_l1w[k * P:(k + 1) * P, :])
                l1w.append(w)
            l2w = cstT.tile([4 * C, C], BF16, tag="l2w")
            nc.sync.dma_start(out=l2w[:], in_=t_l2w[:])
            l1b = cstT.tile([4 * C, 1], F32, tag="l1b")
            nc.sync.dma_start(out=l1b[:], in_=t_l1b[:])
            l2b = cstT.tile([P, C], F32, tag="l2b")
            nc.sync.dma_start(out=l2b[:], in_=t_l2b[:])
            bcat = cst.tile([P, 2, C], F32, tag="bcat")
            nc.sync.dma_start(out=bcat[:].rearrange("p u c -> p (u c)"),
                              in_=t_bcat[:])
            nl16 = cst.tile([P, 1], F32, tag="nl16")
            nc.gpsimd.memset(nl16[:], -LN16)
            erel = cst.tile([P, G * ch], BF16, tag="erel")
            nc.sync.dma_start(out=erel[:], in_=t_erel[:])
            cnt = cst.tile([P, G * ncalls], mybir.dt.int32, tag="cnt")
            nc.sync.dma_start(out=cnt[:], in_=t_cnt[:])
            adn = cst.tile([P, G, 2 * H], BF16, tag="adn")

            # ---- Phase S: self path + Phase A: own-node a_d ----
            with tc.tile_pool(name="psS", bufs=2, space="PSUM") as psS, \
                 tc.tile_pool(name="psT", bufs=2, space="PSUM") as psT:
                blk = 0
                while "S" in phases and blk < cfg.npc:
                    bs = min(512, cfg.npc - blk)
                    x1p = psS.tile([4 * C, 512], F32, tag="x1p")
                    xk = []
                    for k in range(cfg.kd):
                        xts = xtp.tile([P, 512], BF16, tag="xts")
                        nc.sync.dma_start(out=xts[:, :bs],
                                          in_=t_xTs[k * P:(k + 1) * P, blk:blk + bs])
                        xk.append(xts)
                        nc.tensor.matmul(out=x1p[:, :bs], lhsT=l1w[k][:],
                                         rhs=xts[:, :bs],
                                         start=(k == 0), stop=(k == cfg.kd - 1))
                    x1s = wkp.tile([4 * C, 512], BF16, tag="x1s")
                    nc.vector.tensor_add(out=x1s[:, :bs], in0=x1p[:, :bs],
                                         in1=l1b[:].to_broadcast([4 * C, bs]))
                    _elu(nc, wkp, x1s[:, :bs], x1s[:, :bs], "se")
                    for m in range(bs // P):
                        gi = (blk + m * P) // P
                        o2p = psT.tile([P, C], F32, tag="o2p")
                        nc.tensor.matmul(out=o2p[:],
                                         lhsT=x1s[:, m * P:(m + 1) * P],
                                         rhs=l2w[:], start=True, stop=True)
                        o2s = wkp.tile([P, C], F32, tag="o2s")
                        nc.vector.tensor_add(out=o2s[:], in0=o2p[:], in1=l2b[:])
                        _elu(nc, wkp, o2s[:], o2s[:], "so")
                        nc.sync.dma_start(
                            out=t_oself[blk + m * P:blk + (m + 1) * P, :],
                            in_=o2s[:])
                        # own-node a_d for this 128-node group
                        adp = psT.tile([P, 2 * H], F32, tag="adp")
                        for k in range(cfg.kd):
                            nc.tensor.matmul(out=adp[:],
                                             lhsT=xk[k][:, m * P:(m + 1) * P],
                                             rhs=vd[k][:],
                                             start=(k == 0), stop=(k == cfg.kd - 1))
                        nc.vector.tensor_copy(out=adn[:, gi, :], in_=adp[:])
                    blk += bs

            # ---- Phase T: projection tables ----
            with tc.tile_pool(name="psA", bufs=2, space="PSUM") as psA:
                for ts4 in (range(0, cfg.nt, 4) if "T" in phases else []):
                    mt = min(4, cfg.nt - ts4)
                    xk = []
                    for k in range(cfg.kd):
                        xt = xtp.tile([P, 512], BF16, tag=f"xt{k}")
                        nc.sync.dma_start(
                            out=xt[:, :mt * P],
                            in_=t_xT[k * P:(k + 1) * P, ts4 * P:(ts4 + mt) * P])
                        xk.append(xt)
                    for m in range(mt):
                        ts = ts4 + m
                        pt = psA.tile([P, cfg.used], F32, tag="pt")
                        for k in range(cfg.kd):
                            for q0, q1 in ((0, 512), (512, 1024), (1024, 1088)):
                                nc.tensor.matmul(out=pt[:, q0:q1],
                                                 lhsT=xk[k][:, m * P:(m + 1) * P],
                                                 rhs=wcat[k][:, q0:q1],
                                                 start=(k == 0),
                                                 stop=(k == cfg.kd - 1))
                        # table row: [xp1_cm 512 | xp2_cm 512 | a_s1 32 |
                        #             a_s2 32 | pad 64]
                        stag = tbp.tile([P, ROWE], BF16, tag="stag")
                        nc.vector.tensor_copy(
                            out=stag[:, 0:256], in_=pt[:, 0:256])
                        nc.scalar.activation(
                            stag[:, 256:1088], pt[:, 256:1088], Act.Copy)
                        if ts < cfg.ntl:
                            rows = t_tabL[ts * P:(ts + 1) * P, :]
                        else:
                            rows = t_tabH[(ts - cfg.ntl) * P:(ts - cfg.ntl + 1) * P, :]
                        nc.sync.dma_start(out=rows, in_=stag[:])

            stc.close()

            # ---- Phase E: edge aggregation ----
            XB = 3
            with tc.tile_pool(name="gat", bufs=XB) as gat, \
                 tc.tile_pool(name="stp", bufs=2) as stp, \
                 tc.tile_pool(name="wsp", bufs=2) as wsp, \
                 tc.tile_pool(name="psE", bufs=2, space="PSUM") as psE, \
                 tc.tile_pool(name="psD", bufs=2, space="PSUM") as psD, \
                 nc.gpsimd.register("gcnt") as greg:
                npair = ch
                for gi in (range(G) if "E" in phases else []):
                    isrc = gat.tile([P, ch * 8], I16, tag="isrc")
                    nc.sync.dma_start(out=isrc[:],
                                      in_=t_esrc[:, gi * ch * 8:(gi + 1) * ch * 8])
                    sT = stp.tile([P, ch * P], BF16, tag="sT")
                    nc.sync.dma_start(out=sT[:],
                                      in_=t_sT[:, gi * ch * P:(gi + 1) * ch * P])
                    xpa = gat.tile([P, ch, ROWE], BF16, tag="xpa")
                    if gi < XB:
                        nc.gpsimd.memset(xpa[:], 0.0)
                    for ci, (c0, sec) in enumerate(windows):
                        nd = nidx[gi][ci]
                        if nd == 0:
                            continue
                        tab = t_tabL if sec == 0 else t_tabH
                        jw = (nd + P - 1) // P
                        nc.gpsimd.reg_load(
                            greg, cnt[0:1, gi * ncalls + ci:
                                      gi * ncalls + ci + 1])
                        nc.gpsimd.dma_gather(
                            out_ap=xpa[:, c0:c0 + jw, :], in_ap=tab[:],
                            idxs_ap=isrc[:, c0 * 8:c0 * 8 + nd // 16],
                            num_idxs=nd, num_idxs_reg=greg,
                            elem_size=ROWE)

                    out_ps = psE.tile([P, cfg.used], F32, tag="out_ps")
                    pair = 0
                    for c0 in range(0, ch, SUB):
                        jw = min(SUB, ch - c0)
                        # per-edge a_d via one-hot matmul from streamed S_T
                        ade = psD.tile([P, SUB, 2 * H], F32, tag="ade")
                        for j in range(jw):
                            nc.tensor.matmul(
                                out=ade[:, j, :],
                                lhsT=sT[:, (c0 + j) * P:(c0 + j + 1) * P],
                                rhs=adn[:, gi, :], start=True, stop=True)
                        # scores: z = a_s + a_d, leaky, exp(z - ln16)
                        scr = wsp.tile([P, SUB, 2 * H], BF16, tag="scr")
                        nc.vector.tensor_add(
                            out=scr[:, :jw, :].rearrange(
                                "p s (u t) -> p s u t", u=2),
                            in0=xpa[:, c0:c0 + jw, 1024:1088].rearrange(
                                "p s (u t) -> p s u t", u=2),
                            in1=ade[:, :jw, :].rearrange(
                                "p s (u t) -> p s u t", u=2))
                        nc.vector.scalar_tensor_tensor(
                            out=scr[:, :jw, :], in0=scr[:, :jw, :], scalar=0.2,
                            in1=scr[:, :jw, :], op0=Alu.mult, op1=Alu.max)
                        wsc = wsp.tile([P, SUB, cfg.used], BF16, tag="wsc")
                        nc.scalar.activation(
                            wsc[:, :jw, 1024:1088], scr[:, :jw, :], Act.Exp,
                            bias=nl16[:], scale=1.0)
                        # weighted features (c-major: bcast along inner h)
                        for i in range(2):
                            nc.vector.tensor_tensor(
                                out=wsc[:, :jw, i * HD:(i + 1) * HD].rearrange(
                                    "p s (c h) -> p s c h", c=C),
                                in0=xpa[:, c0:c0 + jw,
                                        i * HD:(i + 1) * HD].rearrange(
                                    "p s (c h) -> p s c h", c=C),
                                in1=wsc[:, :jw, 1024 + i * H:1024 + (i + 1) * H]
                                    .unsqueeze(2).to_broadcast([P, jw, C, H]),
                                op=Alu.mult)
                        # one-hot S for this span
                        S = wsp.tile([P, SUB, P], BF16, tag="S")
                        nc.vector.tensor_tensor(
                            out=S[:, :jw, :],
                            in0=erel[:, gi * ch + c0:gi * ch + c0 + jw]
                                .unsqueeze(2).to_broadcast([P, jw, P]),
                            in1=iota_b[:].unsqueeze(1).to_broadcast([P, jw, P]),
                            op=Alu.is_equal)
                        for jj in range(jw):
                            for q0, q1 in ((0, 512), (512, 1024), (1024, 1088)):
                                nc.tensor.matmul(
                                    out=out_ps[:, q0:q1],
                                    lhsT=S[:, jj, :],
                                    rhs=wsc[:, jj, q0:q1],
                                    start=(pair == 0), stop=(pair == npair - 1))
                            pair += 1
                    # normalize + head mean (inner) + bias + elu
                    rsb = wkp.tile([P, 2 * H], F32, tag="rsb")
                    nc.scalar.activation(rsb[:], out_ps[:, 1024:1088],
                                         Act.Copy, scale=float(H), bias=1e-12)
                    nc.vector.reciprocal(rsb[:], rsb[:])
                    tmp = wkp.tile([P, 2, C, H], F32, tag="tmp")
                    nc.vector.tensor_tensor(
                        out=tmp[:],
                        in0=out_ps[:, 0:1024].rearrange(
                            "p (u c h) -> p u c h", u=2, c=C),
                        in1=rsb[:].rearrange("p (u h) -> p u h", u=2)
                            .unsqueeze(2).to_broadcast([P, 2, C, H]),
                        op=Alu.mult)
                    om = wkp.tile([P, 2, C], F32, tag="om")
                    nc.vector.reduce_sum(out=om[:], in_=tmp[:], axis=AX_X)
                    nc.vector.tensor_add(out=om[:], in0=om[:], in1=bcat[:])
                    _elu(nc, wkp, om[:], om[:], "oe")
                    for i in range(2):
                        nc.sync.dma_start(
                            out=t_oconv[i][gi * P:(gi + 1) * P, :],
                            in_=om[:, i, :])
    nc.compile()
    return nc


def _wrap16(flat):
    """edge i -> [i%16, i//16], replicated to 128 partitions."""
    w = flat.reshape(-1, 16).T  # [16, len/16]
    return np.tile(w, (8, 1))


def preprocess(cfg: Cfg, inputs):
    n, H, C, G = cfg.n, cfg.h, cfg.c, cfg.groups
    x = np.asarray(inputs["x"], np.float32)
    ei = np.asarray(inputs["edge_index"])

    def fold(W, a_s, a_d):
        W = np.asarray(W, np.float32).reshape(cfg.d_in, H, C)
        v_s = (W * np.asarray(a_s, np.float32)[None]).sum(-1)
        v_d = (W * np.asarray(a_d, np.float32)[None]).sum(-1)
        # c-major projection columns: col (c*H + h) = W[:, h, c]
        W_cm = W.transpose(0, 2, 1).reshape(cfg.d_in, H * C)
        return W_cm, v_s, v_d

    W1, vs1, vd1 = fold(inputs["W1"], inputs["att_src1"], inputs["att_dst1"])
    W2, vs2, vd2 = fold(inputs["W2"], inputs["att_src2"], inputs["att_dst2"])
    # psum col layout: [xp1_cm 512 | xp2_cm 512 | a_s1 32 | a_s2 32]
    wcat = np.concatenate([W1, W2, vs1, vs2], 1).astype(ml_dtypes.bfloat16)
    vdcat = np.concatenate([vd1, vd2], 1).astype(ml_dtypes.bfloat16)

    xT = np.zeros((cfg.d_in, cfg.n_pad), ml_dtypes.bfloat16)
    xT[:, :n] = x.T.astype(ml_dtypes.bfloat16)

    loops = np.arange(n, dtype=np.int64)
    src = np.concatenate([ei[0], loops]).astype(np.int32)
    dst = np.concatenate([ei[1], loops]).astype(np.int32)
    order = np.argsort(dst, kind="stable")
    src_s = src[order]
    dst_s = dst[order]
    dev = dst_s // cfg.npc
    rel = dst_s - dev * cfg.npc
    grp = rel >> 7
    hi_e = (src_s >= cfg.lo).astype(np.int64)
    sidx = (dev * G + grp) * 2 + hi_e
    counts = np.bincount(sidx, minlength=cfg.n_cores * G * 2)
    c2 = counts.reshape(-1, 2)

    ch_lo = max(1, int(math.ceil(c2[:, 0].max() / P)))
    ch_hi = max(1, int(math.ceil(c2[:, 1].max() / P)))
    ch = ch_lo + ch_hi
    order2 = np.argsort(sidx, kind="stable")
    src2 = src_s[order2]
    sidx2 = sidx[order2]
    rel2 = (rel & 127)[order2]
    starts = np.zeros(len(counts), np.int64)
    starts[1:] = np.cumsum(counts)[:-1]
    pos_in_sec = np.arange(len(src2)) - starts[sidx2]
    dev2 = sidx2 // (G * 2)
    grp2 = (sidx2 // 2) % G
    hi2 = sidx2 & 1
    slot = np.where(hi2 == 0, pos_in_sec, ch_lo * P + pos_in_sec)
    gpos = grp2 * (ch * P) + slot

    GE = G * ch * P
    src16 = np.full((cfg.n_cores, GE), -1, np.int16)
    relf = np.full((cfg.n_cores, GE), -1.0, np.float32)
    sTf = np.zeros((cfg.n_cores, P, GE), ml_dtypes.bfloat16)
    src_adj = np.where(hi2 == 1, src2 - cfg.lo, src2).astype(np.int16)
    src16[dev2, gpos] = src_adj
    relf[dev2, gpos] = rel2.astype(np.float32)
    sTf[dev2, rel2, gpos] = 1.0

    # per-(core, group, section) real counts
    secn = np.zeros((cfg.n_cores, G, 2), np.int64)
    np.add.at(secn, (dev2, grp2, hi2), 1)
    # gather call windows: gcall-chunk slices of each section
    GC = cfg.gcall
    calls_lo = math.ceil(ch_lo / GC)
    calls_hi = math.ceil(ch_hi / GC)
    ncalls = calls_lo + calls_hi
    cnts = np.zeros((cfg.n_cores, G, ncalls), np.int32)
    nidx = []
    nmax = np.maximum(secn.max(axis=0), 1)          # [G, 2]
    for ci in range(ncalls):
        if ci < calls_lo:
            s, base = 0, ci * GC * P
            width = min(GC, ch_lo - ci * GC) * P
        else:
            s, base = 1, (ci - calls_lo) * GC * P
            width = min(GC, ch_hi - (ci - calls_lo) * GC) * P
        real = np.clip(secn[:, :, s] - base, 0, width)        # [cores, G]
        stat = np.clip(-(-(nmax[:, s] - base) // P) * P, 0, width)  # [G]
        cnts[:, :, ci] = np.where(real > 0, real, 1)
        nidx.append(stat)
        # sentinel index 0 for cores with an empty (but emitted) window
        need = (real == 0) & (stat[None, :] > 0)
        if need.any():
            cc, gg = np.nonzero(need)
            sec_off = 0 if s == 0 else ch_lo * P
            src16[cc, gg * (ch * P) + sec_off + base] = 0
    nidx = tuple(tuple(int(nidx[ci][g]) for ci in range(ncalls))
                 for g in range(G))

    l1b = np.asarray(inputs["lin1_b"], np.float32).reshape(4 * C, 1)
    l2b = np.broadcast_to(np.asarray(inputs["lin2_b"], np.float32), (P, C)).copy()
    bcat = np.broadcast_to(
        np.concatenate([np.asarray(inputs["b1"], np.float32),
                        np.asarray(inputs["b2"], np.float32)]), (P, 2 * C)).copy()

    in_maps = []
    for c in range(cfg.n_cores):
        xTs = np.zeros((cfg.d_in, cfg.npc), ml_dtypes.bfloat16)
        lo_n = min(cfg.npc, max(0, n - c * cfg.npc))
        xTs[:, :lo_n] = xT[:, c * cfg.npc:c * cfg.npc + lo_n]
        esrc = np.concatenate(
            [_wrap16(src16[c, gi * ch * P:(gi + 1) * ch * P])
             for gi in range(G)], axis=1)
        erel = np.concatenate(
            [relf[c, gi * ch * P:(gi + 1) * ch * P].reshape(ch, P).T
             for gi in range(G)], axis=1).astype(ml_dtypes.bfloat16)
        cntc = np.broadcast_to(cnts[c].reshape(1, -1),
                               (P, cnts.shape[1] * cnts.shape[2])).copy()
        in_maps.append({
            "xT": xT, "xTs": xTs, "wcat": wcat, "vd": vdcat, "cnt": cntc,
            "l1w": np.asarray(inputs["lin1_w"], np.float32).astype(
                ml_dtypes.bfloat16),
            "l2w": np.asarray(inputs["lin2_w"], np.float32).astype(
                ml_dtypes.bfloat16),
            "l1b": l1b, "l2b": l2b, "bcat": bcat,
            "esrc": np.ascontiguousarray(esrc),
            "erel": np.ascontiguousarray(erel),
            "sT": np.ascontiguousarray(sTf[c]),
        })
    return in_maps, ch_lo, ch_hi, nidx


_CACHE = {}


def kernel(**inputs):
    cfg = Cfg()
    in_maps, ch_lo, ch_hi, nidx = preprocess(cfg, inputs)
    key = ("v5", ch_lo, ch_hi, nidx)
    if key not in _CACHE:
        _CACHE[key] = build_program(cfg, ch_lo, ch_hi, nidx)
    nc = _CACHE[key]
    res = run_bass_kernel_spmd(nc, in_maps, list(range(cfg.n_cores))).results
    takes = [min(cfg.npc, cfg.n - c * cfg.npc) for c in range(cfg.n_cores)]
    x_in = np.concatenate([res[c]["out_conv0"][:takes[c]]
                           for c in range(cfg.n_cores)])
    x_out = np.concatenate([res[c]["out_conv1"][:takes[c]]
                            for c in range(cfg.n_cores)])
    x_self = np.concatenate([res[c]["out_self"][:takes[c]]
                             for c in range(cfg.n_cores)])
    return (x_in, x_out, x_self)


# revision 16
# speedup vs baseline: 1.4596x; 1.0178x over previous
"""Distributed GAT (2x GATConv + MLP self-path) on 8 Trainium2 NeuronCores.

Strategy (dst-node graph parallelism, SPMD on 8 cores, v3):
  Host:
    - fold attention vectors into projection weights, cast x/weights to bf16,
      append self-loops, sort edges by dst, partition edges by 6272-node
      (128-aligned) dst blocks per core, group each core's dst nodes into
      128-node groups, order each group's edges [src<LO | src>=LO] with both
      sections padded to chunk counts ch_lo/ch_hi (global constants so the
      SPMD program is identical across cores); pad gather indices with -1.
    - xp table columns are C-MAJOR per conv ([c*32+h]) so the per-edge
      per-head weighting broadcasts along the INNER head axis (DVE 2x mode).
    - stream the per-chunk transposed one-hot dst-selector S_T (bf16) so the
      per-edge a_d term becomes a tiny on-device matmul instead of a gather.
    - per-(group,call) gather num_idxs statically trimmed to the max real
      count over the 8 cores (desc-gen on the Q7 costs ~10-18ns per padded
      slot, so padding is expensive).
  Device:
    - Phase S: MLP self path (bf16 matmuls) on own node block.
    - Phase A: a_d for own dst nodes from x_own @ v_d (kept in SBUF).
    - Phase T: projection table row(n) = [xp1_cm 512|xp2_cm 512|a_s1 32|
      a_s2 32|pad 64] bf16 (2304 B) written as two tables (node < LO /
      >= LO so int16 gather indices reach every row).
    - Phase E: per 128-dst-node group: one dma_gather per section for all
      edges (both convs share the row), scores exp(leakyrelu(a_s+a_d)-ln16)
      without max-subtraction (exact softmax identity, scale cancels),
      one-hot segment matmuls accumulate the weighted feature sum and the
      softmax denominator in one PSUM tile; normalisation, head-mean (inner
      reduce in c-major), bias, elu.
  Host: concatenate per-core dst blocks.
"""
import math
import numpy as np
import ml_dtypes

import concourse.bass as bass
import concourse.tile as tile
import concourse.mybir as mybir
import bass_rust
from concourse import bacc
from concourse.bass_utils import run_bass_kernel_spmd

AX_X = bass_rust.AxisListType.X
F32 = mybir.dt.float32
BF16 = mybir.dt.bfloat16
I16 = mybir.dt.int16
Act = mybir.ActivationFunctionType
Alu = mybir.AluOpType
P = 128
LN16 = float(np.log(16.0))


class Cfg:
    def __init__(self):
        self.n = 50000
        self.d_in = 256
        self.c = 16
        self.h = 32
        self.hd = self.h * self.c           # 512
        self.n_cores = 8
        self.npc = 6272                     # 128-aligned dst block per core
        self.groups = self.npc // P         # 49
        self.n_pad = self.npc * self.n_cores  # 50176
        self.nt = self.n_pad // P           # 392 node tiles
        self.kd = self.d_in // P            # 2
        self.lo = 32768
        self.ntl = self.lo // P             # 256 tiles in lo table
        self.n_hi = self.n_pad - self.lo    # 17408
        self.rowe = 1152                    # table row bf16 elements (w/ pad)
        self.used = 1088                    # meaningful row columns
        self.sub = 4                        # chunks per span
        self.gcall = 8                      # chunks per gather call (<=64
                                            # descriptors per SDMA engine)


def _elu(nc, pool, out_ap, in_ap, tag):
    shape = list(in_ap.shape)
    u = pool.tile(shape, F32, tag=tag + "_u")
    rl = pool.tile(shape, F32, tag=tag + "_r")
    nc.vector.tensor_scalar_min(out=u[:], in0=in_ap, scalar1=0.0)
    nc.scalar.activation(u[:], u[:], Act.Exp)
    nc.scalar.activation(rl[:], in_ap, Act.Relu)
    nc.vector.scalar_tensor_tensor(
        out=out_ap, in0=u[:], scalar=-1.0, in1=rl[:], op0=Alu.add, op1=Alu.add)


def build_program(cfg: Cfg, ch_lo: int, ch_hi: int, nidx, phases: str = "STE"):
    """nidx: per-group tuple of per-call static gather num_idxs (128-aligned,
    maxed over cores, <=1024 each; 0 = skip the call). Call windows are
    gcall-chunk slices of the lo then hi sections."""
    nc = bacc.Bacc("TRN2", target_bir_lowering=False, debug=False,
                   num_devices=cfg.n_cores)
    G, H, C, HD = cfg.groups, cfg.h, cfg.c, cfg.hd
    ch = ch_lo + ch_hi
    ROWE = cfg.rowe
    SUB = cfg.sub
    GC = cfg.gcall
    calls_lo = math.ceil(ch_lo / GC)
    calls_hi = math.ceil(ch_hi / GC)
    ncalls = calls_lo + calls_hi
    # (chunk base, table) per call window
    windows = [(ci * GC, 0) for ci in range(calls_lo)] + \
              [(ch_lo + ci * GC, 1) for ci in range(calls_hi)]

    t_xT = nc.dram_tensor("xT", [cfg.d_in, cfg.n_pad], BF16, kind="ExternalInput")
    t_xTs = nc.dram_tensor("xTs", [cfg.d_in, cfg.npc], BF16, kind="ExternalInput")
    t_wcat = nc.dram_tensor("wcat", [cfg.d_in, cfg.used], BF16, kind="ExternalInput")
    t_vd = nc.dram_tensor("vd", [cfg.d_in, 2 * H], BF16, kind="ExternalInput")
    t_l1w = nc.dram_tensor("l1w", [cfg.d_in, 4 * C], BF16, kind="ExternalInput")
    t_l2w = nc.dram_tensor("l2w", [4 * C, C], BF16, kind="ExternalInput")
    t_l1b = nc.dram_tensor("l1b", [4 * C, 1], F32, kind="ExternalInput")
    t_l2b = nc.dram_tensor("l2b", [P, C], F32, kind="ExternalInput")
    t_bcat = nc.dram_tensor("bcat", [P, 2 * C], F32, kind="ExternalInput")
    t_esrc = nc.dram_tensor("esrc", [P, G * ch * 8], I16, kind="ExternalInput")
    t_cnt = nc.dram_tensor("cnt", [P, G * ncalls], mybir.dt.int32,
                           kind="ExternalInput")
    t_erel = nc.dram_tensor("erel", [P, G * ch], BF16, kind="ExternalInput")
    t_sT = nc.dram_tensor("sT", [P, G * ch * P], BF16, kind="ExternalInput")

    t_oconv = [nc.dram_tensor(f"out_conv{i}", [cfg.npc, C], F32,
                              kind="ExternalOutput") for i in range(2)]
    t_oself = nc.dram_tensor("out_self", [cfg.npc, C], F32,
                             kind="ExternalOutput")

    t_tabL = nc.dram_tensor("tabL", [cfg.lo, ROWE], BF16)
    t_tabH = nc.dram_tensor("tabH", [cfg.n_hi, ROWE], BF16)

    with tile.TileContext(nc) as tc:
        import contextlib
        with contextlib.ExitStack() as ctx:
            cst = ctx.enter_context(tc.tile_pool(name="cst", bufs=1))
            wkp = ctx.enter_context(tc.tile_pool(name="wkp", bufs=2))
            stc = contextlib.ExitStack()
            cstT = stc.enter_context(tc.tile_pool(name="cstT", bufs=1))
            xtp = stc.enter_context(tc.tile_pool(name="xtp", bufs=3))
            tbp = stc.enter_context(tc.tile_pool(name="tbp", bufs=3))

            # ---- constants ----
            iota_i = cst.tile([P, P], mybir.dt.int32, tag="ioi")
            nc.gpsimd.iota(iota_i[:], pattern=[[1, P]], base=0, channel_multiplier=0)
            iota_b = cst.tile([P, P], BF16, tag="iob")
            nc.vector.tensor_copy(iota_b[:], iota_i[:])
            wcat = []
            for k in range(cfg.kd):
                w = cstT.tile([P, cfg.used], BF16, tag=f"wc{k}")
                nc.sync.dma_start(out=w[:], in_=t_wcat[k * P:(k + 1) * P, :])
                wcat.append(w)
            vd = []
            for k in range(cfg.kd):
                w = cstT.tile([P, 2 * H], BF16, tag=f"vd{k}")
                nc.sync.dma_start(out=w[:], in_=t_vd[k * P:(k + 1) * P, :])
                vd.append(w)
            l1w = []
            for k in range(cfg.kd):
                w = cstT.tile([P, 4 * C], BF16, tag=f"l1w{k}")
                nc.sync.dma_start(out=w[:], in_=t_l1w[k * P:(k + 1) * P, :])
                l1w.append(w)
            l2w = cstT.tile([4 * C, C], BF16, tag="l2w")
            nc.sync.dma_start(out=l2w[:], in_=t_l2w[:])
            l1b = cstT.tile([4 * C, 1], F32, tag="l1b")
            nc.sync.dma_start(out=l1b[:], in_=t_l1b[:])
            l2b = cstT.tile([P, C], F32, tag="l2b")
            nc.sync.dma_start(out=l2b[:], in_=t_l2b[:])
            bcat = cst.tile([P, 2, C], F32, tag="bcat")
            nc.sync.dma_start(out=bcat[:].rearrange("p u c -> p (u c)"),
                              in_=t_bcat[:])
            nl16 = cst.tile([P, 1], F32, tag="nl16")
            nc.gpsimd.memset(nl16[:], -LN16)
            erel = cst.tile([P, G * ch], BF16, tag="erel")
            nc.sync.dma_start(out=erel[:], in_=t_erel[:])
            cnt = cst.tile([P, G * ncalls], mybir.dt.int32, tag="cnt")
            nc.sync.dma_start(out=cnt[:], in_=t_cnt[:])
            adn = cst.tile([P, G, 2 * H], BF16, tag="adn")

            # ---- Phase S: self path + Phase A: own-node a_d ----
            with tc.tile_pool(name="psS", bufs=2, space="PSUM") as psS, \
                 tc.tile_pool(name="psT", bufs=2, space="PSUM") as psT:
                blk = 0
                while "S" in phases and blk < cfg.npc:
                    bs = min(512, cfg.npc - blk)
                    x1p = psS.tile([4 * C, 512], F32, tag="x1p")
                    xk = []
                    for k in range(cfg.kd):
                        xts = xtp.tile([P, 512], BF16, tag="xts")
                        nc.sync.dma_start(out=xts[:, :bs],
                                          in_=t_xTs[k * P:(k + 1) * P, blk:blk + bs])
                        xk.append(xts)
                        nc.tensor.matmul(out=x1p[:, :bs], lhsT=l1w[k][:],
                                         rhs=xts[:, :bs],
                                         start=(k == 0), stop=(k == cfg.kd - 1))
                    x1s = wkp.tile([4 * C, 512], BF16, tag="x1s")
                    nc.vector.tensor_add(out=x1s[:, :bs], in0=x1p[:, :bs],
                                         in1=l1b[:].to_broadcast([4 * C, bs]))
                    _elu(nc, wkp, x1s[:, :bs], x1s[:, :bs], "se")
                    for m in range(bs // P):
                        gi = (blk + m * P) // P
                        o2p = psT.tile([P, C], F32, tag="o2p")
                        nc.tensor.matmul(out=o2p[:],
                                         lhsT=x1s[:, m * P:(m + 1) * P],
                                         rhs=l2w[:], start=True, stop=True)
                        o2s = wkp.tile([P, C], F32, tag="o2s")
                        nc.vector.tensor_add(out=o2s[:], in0=o2p[:], in1=l2b[:])
                        _elu(nc, wkp, o2s[:], o2s[:], "so")
                        nc.sync.dma_start(
                            out=t_oself[blk + m * P:blk + (m + 1) * P, :],
                            in_=o2s[:])
                        # own-node a_d for this 128-node group
                        adp = psT.tile([P, 2 * H], F32, tag="adp")
                        for k in range(cfg.kd):
                            nc.tensor.matmul(out=adp[:],
                                             lhsT=xk[k][:, m * P:(m + 1) * P],
                                             rhs=vd[k][:],
                                             start=(k == 0), stop=(k == cfg.kd - 1))
                        nc.vector.tensor_copy(out=adn[:, gi, :], in_=adp[:])
                    blk += bs

            # ---- Phase T: projection tables ----
            with tc.tile_pool(name="psA", bufs=2, space="PSUM") as psA:
                for ts4 in (range(0, cfg.nt, 8) if "T" in phases else []):
                    mt = min(8, cfg.nt - ts4)
                    xk = []
                    for k in range(cfg.kd):
                        xt = xtp.tile([P, 1024], BF16, tag=f"xt{k}")
                        nc.sync.dma_start(
                            out=xt[:, :mt * P],
                            in_=t_xT[k * P:(k + 1) * P, ts4 * P:(ts4 + mt) * P])
                        xk.append(xt)
                    for m in range(mt):
                        ts = ts4 + m
                        pt = psA.tile([P, cfg.used], F32, tag="pt")
                        for k in range(cfg.kd):
                            for q0, q1 in ((0, 512), (512, 1024), (1024, 1088)):
                                nc.tensor.matmul(out=pt[:, q0:q1],
                                                 lhsT=xk[k][:, m * P:(m + 1) * P],
                                                 rhs=wcat[k][:, q0:q1],
                                                 start=(k == 0),
                                                 stop=(k == cfg.kd - 1))
                        # table row: [xp1_cm 512 | xp2_cm 512 | a_s1 32 |
                        #             a_s2 32 | pad 64]
                        stag = tbp.tile([P, ROWE], BF16, tag="stag")
                        nc.vector.tensor_copy(
                            out=stag[:, 0:256], in_=pt[:, 0:256])
                        nc.scalar.activation(
                            stag[:, 256:1088], pt[:, 256:1088], Act.Copy)
                        if ts < cfg.ntl:
                            rows = t_tabL[ts * P:(ts + 1) * P, :]
                        else:
                            rows = t_tabH[(ts - cfg.ntl) * P:(ts - cfg.ntl + 1) * P, :]
                        nc.sync.dma_start(out=rows, in_=stag[:])

            stc.close()

            # ---- Phase E: edge aggregation ----
            XB = 3
            with tc.tile_pool(name="gat", bufs=XB) as gat, \
                 tc.tile_pool(name="stp", bufs=2) as stp, \
                 tc.tile_pool(name="wsp", bufs=2) as wsp, \
                 tc.tile_pool(name="psE", bufs=2, space="PSUM") as psE, \
                 tc.tile_pool(name="psD", bufs=2, space="PSUM") as psD, \
                 nc.gpsimd.register("gcnt") as greg:
                npair = ch
                for gi in (range(G) if "E" in phases else []):
                    isrc = gat.tile([P, ch * 8], I16, tag="isrc")
                    nc.sync.dma_start(out=isrc[:],
                                      in_=t_esrc[:, gi * ch * 8:(gi + 1) * ch * 8])
                    sT = stp.tile([P, ch * P], BF16, tag="sT")
                    nc.sync.dma_start(out=sT[:],
                                      in_=t_sT[:, gi * ch * P:(gi + 1) * ch * P])
                    xpa = gat.tile([P, ch, ROWE], BF16, tag="xpa")
                    if gi < XB:
                        nc.gpsimd.memset(xpa[:], 0.0)
                    for ci, (c0, sec) in enumerate(windows):
                        nd = nidx[gi][ci]
                        if nd == 0:
                            continue
                        tab = t_tabL if sec == 0 else t_tabH
                        jw = (nd + P - 1) // P
                        nc.gpsimd.reg_load(
                            greg, cnt[0:1, gi * ncalls + ci:
                                      gi * ncalls + ci + 1])
                        nc.gpsimd.dma_gather(
                            out_ap=xpa[:, c0:c0 + jw, :], in_ap=tab[:],
                            idxs_ap=isrc[:, c0 * 8:c0 * 8 + nd // 16],
                            num_idxs=nd, num_idxs_reg=greg,
                            elem_size=ROWE)

                    out_ps = psE.tile([P, cfg.used], F32, tag="out_ps")
                    pair = 0
                    for c0 in range(0, ch, SUB):
                        jw = min(SUB, ch - c0)
                        # per-edge a_d via one-hot matmul from streamed S_T
                        ade = psD.tile([P, SUB, 2 * H], F32, tag="ade")
                        for j in range(jw):
                            nc.tensor.matmul(
                                out=ade[:, j, :],
                                lhsT=sT[:, (c0 + j) * P:(c0 + j + 1) * P],
                                rhs=adn[:, gi, :], start=True, stop=True)
                        # scores: z = a_s + a_d, leaky, exp(z - ln16)
                        scr = wsp.tile([P, SUB, 2 * H], BF16, tag="scr")
                        nc.vector.tensor_add(
                            out=scr[:, :jw, :].rearrange(
                                "p s (u t) -> p s u t", u=2),
                            in0=xpa[:, c0:c0 + jw, 1024:1088].rearrange(
                                "p s (u t) -> p s u t", u=2),
                            in1=ade[:, :jw, :].rearrange(
                                "p s (u t) -> p s u t", u=2))
                        nc.vector.scalar_tensor_tensor(
                            out=scr[:, :jw, :], in0=scr[:, :jw, :], scalar=0.2,
                            in1=scr[:, :jw, :], op0=Alu.mult, op1=Alu.max)
                        wsc = wsp.tile([P, SUB, cfg.used], BF16, tag="wsc")
                        nc.scalar.activation(
                            wsc[:, :jw, 1024:1088], scr[:, :jw, :], Act.Exp,
                            bias=nl16[:], scale=1.0)
                        # weighted features (c-major: bcast along inner h)
                        for i in range(2):
                            nc.vector.tensor_tensor(
                                out=wsc[:, :jw, i * HD:(i + 1) * HD].rearrange(
                                    "p s (c h) -> p s c h", c=C),
                                in0=xpa[:, c0:c0 + jw,
                                        i * HD:(i + 1) * HD].rearrange(
                                    "p s (c h) -> p s c h", c=C),
                                in1=wsc[:, :jw, 1024 + i * H:1024 + (i + 1) * H]
                                    .unsqueeze(2).to_broadcast([P, jw, C, H]),
                                op=Alu.mult)
                        # one-hot S for this span
                        S = wsp.tile([P, SUB, P], BF16, tag="S")
                        nc.vector.tensor_tensor(
                            out=S[:, :jw, :],
                            in0=erel[:, gi * ch + c0:gi * ch + c0 + jw]
                                .unsqueeze(2).to_broadcast([P, jw, P]),
                            in1=iota_b[:].unsqueeze(1).to_broadcast([P, jw, P]),
                            op=Alu.is_equal)
                        for jj in range(jw):
                            for q0, q1 in ((0, 512), (512, 1024), (1024, 1088)):
                                nc.tensor.matmul(
                                    out=out_ps[:, q0:q1],
                                    lhsT=S[:, jj, :],
                                    rhs=wsc[:, jj, q0:q1],
                                    start=(pair == 0), stop=(pair == npair - 1))
                            pair += 1
                    # normalize + head mean (inner) + bias + elu
                    rsb = wkp.tile([P, 2 * H], F32, tag="rsb")
                    nc.scalar.activation(rsb[:], out_ps[:, 1024:1088],
                                         Act.Copy, scale=float(H), bias=1e-12)
                    nc.vector.reciprocal(rsb[:], rsb[:])
                    tmp = wkp.tile([P, 2, C, H], F32, tag="tmp")
                    nc.vector.tensor_tensor(
                        out=tmp[:],
                        in0=out_ps[:, 0:1024].rearrange(
                            "p (u c h) -> p u c h", u=2, c=C),
                        in1=rsb[:].rearrange("p (u h) -> p u h", u=2)
                            .unsqueeze(2).to_broadcast([P, 2, C, H]),
                        op=Alu.mult)
                    om = wkp.tile([P, 2, C], F32, tag="om")
                    nc.vector.reduce_sum(out=om[:], in_=tmp[:], axis=AX_X)
                    nc.vector.tensor_add(out=om[:], in0=om[:], in1=bcat[:])
                    _elu(nc, wkp, om[:], om[:], "oe")
                    for i in range(2):
                        nc.sync.dma_start(
                            out=t_oconv[i][gi * P:(gi + 1) * P, :],
                            in_=om[:, i, :])
    nc.compile()
    return nc


def _wrap16(flat):
    """edge i -> [i%16, i//16], replicated to 128 partitions."""
    w = flat.reshape(-1, 16).T  # [16, len/16]
    return np.tile(w, (8, 1))


def preprocess(cfg: Cfg, inputs):
    n, H, C, G = cfg.n, cfg.h, cfg.c, cfg.groups
    x = np.asarray(inputs["x"], np.float32)
    ei = np.asarray(inputs["edge_index"])

    def fold(W, a_s, a_d):
        W = np.asarray(W, np.float32).reshape(cfg.d_in, H, C)
        v_s = (W * np.asarray(a_s, np.float32)[None]).sum(-1)
        v_d = (W * np.asarray(a_d, np.float32)[None]).sum(-1)
        # c-major projection columns: col (c*H + h) = W[:, h, c]
        W_cm = W.transpose(0, 2, 1).reshape(cfg.d_in, H * C)
        return W_cm, v_s, v_d

    W1, vs1, vd1 = fold(inputs["W1"], inputs["att_src1"], inputs["att_dst1"])
    W2, vs2, vd2 = fold(inputs["W2"], inputs["att_src2"], inputs["att_dst2"])
    # psum col layout: [xp1_cm 512 | xp2_cm 512 | a_s1 32 | a_s2 32]
    wcat = np.concatenate([W1, W2, vs1, vs2], 1).astype(ml_dtypes.bfloat16)
    vdcat = np.concatenate([vd1, vd2], 1).astype(ml_dtypes.bfloat16)

    xT = np.zeros((cfg.d_in, cfg.n_pad), ml_dtypes.bfloat16)
    xT[:, :n] = x.T.astype(ml_dtypes.bfloat16)

    loops = np.arange(n, dtype=np.int64)
    src = np.concatenate([ei[0], loops]).astype(np.int32)
    dst = np.concatenate([ei[1], loops]).astype(np.int32)
    order = np.argsort(dst, kind="stable")
    src_s = src[order]
    dst_s = dst[order]
    dev = dst_s // cfg.npc
    rel = dst_s - dev * cfg.npc
    grp = rel >> 7
    hi_e = (src_s >= cfg.lo).astype(np.int64)
    sidx = (dev * G + grp) * 2 + hi_e
    counts = np.bincount(sidx, minlength=cfg.n_cores * G * 2)
    c2 = counts.reshape(-1, 2)

    ch_lo = max(1, int(math.ceil(c2[:, 0].max() / P)))
    ch_hi = max(1, int(math.ceil(c2[:, 1].max() / P)))
    ch = ch_lo + ch_hi
    order2 = np.argsort(sidx, kind="stable")
    src2 = src_s[order2]
    sidx2 = sidx[order2]
    rel2 = (rel & 127)[order2]
    starts = np.zeros(len(counts), np.int64)
    starts[1:] = np.cumsum(counts)[:-1]
    pos_in_sec = np.arange(len(src2)) - starts[sidx2]
    dev2 = sidx2 // (G * 2)
    grp2 = (sidx2 // 2) % G
    hi2 = sidx2 & 1
    slot = np.where(hi2 == 0, pos_in_sec, ch_lo * P + pos_in_sec)
    gpos = grp2 * (ch * P) + slot

    GE = G * ch * P
    src16 = np.full((cfg.n_cores, GE), -1, np.int16)
    relf = np.full((cfg.n_cores, GE), -1.0, np.float32)
    sTf = np.zeros((cfg.n_cores, P, GE), ml_dtypes.bfloat16)
    src_adj = np.where(hi2 == 1, src2 - cfg.lo, src2).astype(np.int16)
    src16[dev2, gpos] = src_adj
    relf[dev2, gpos] = rel2.astype(np.float32)
    sTf[dev2, rel2, gpos] = 1.0

    # per-(core, group, section) real counts
    secn = np.zeros((cfg.n_cores, G, 2), np.int64)
    np.add.at(secn, (dev2, grp2, hi2), 1)
    # gather call windows: gcall-chunk slices of each section
    GC = cfg.gcall
    calls_lo = math.ceil(ch_lo / GC)
    calls_hi = math.ceil(ch_hi / GC)
    ncalls = calls_lo + calls_hi
    cnts = np.zeros((cfg.n_cores, G, ncalls), np.int32)
    nidx = []
    nmax = np.maximum(secn.max(axis=0), 1)          # [G, 2]
    for ci in range(ncalls):
        if ci < calls_lo:
            s, base = 0, ci * GC * P
            width = min(GC, ch_lo - ci * GC) * P
        else:
            s, base = 1, (ci - calls_lo) * GC * P
            width = min(GC, ch_hi - (ci - calls_lo) * GC) * P
        real = np.clip(secn[:, :, s] - base, 0, width)        # [cores, G]
        stat = np.clip(-(-(nmax[:, s] - base) // 16) * 16, 0, width)  # [G]
        cnts[:, :, ci] = np.where(real > 0, real, 1)
        nidx.append(stat)
        # sentinel index 0 for cores with an empty (but emitted) window
        need = (real == 0) & (stat[None, :] > 0)
        if need.any():
            cc, gg = np.nonzero(need)
            sec_off = 0 if s == 0 else ch_lo * P
            src16[cc, gg * (ch * P) + sec_off + base] = 0
    nidx = tuple(tuple(int(nidx[ci][g]) for ci in range(ncalls))
                 for g in range(G))

    l1b = np.asarray(inputs["lin1_b"], np.float32).reshape(4 * C, 1)
    l2b = np.broadcast_to(np.asarray(inputs["lin2_b"], np.float32), (P, C)).copy()
    bcat = np.broadcast_to(
        np.concatenate([np.asarray(inputs["b1"], np.float32),
                        np.asarray(inputs["b2"], np.float32)]), (P, 2 * C)).copy()

    in_maps = []
    for c in range(cfg.n_cores):
        xTs = np.zeros((cfg.d_in, cfg.npc), ml_dtypes.bfloat16)
        lo_n = min(cfg.npc, max(0, n - c * cfg.npc))
        xTs[:, :lo_n] = xT[:, c * cfg.npc:c * cfg.npc + lo_n]
        esrc = np.concatenate(
            [_wrap16(src16[c, gi * ch * P:(gi + 1) * ch * P])
             for gi in range(G)], axis=1)
        erel = np.concatenate(
            [relf[c, gi * ch * P:(gi + 1) * ch * P].reshape(ch, P).T
             for gi in range(G)], axis=1).astype(ml_dtypes.bfloat16)
        cntc = np.broadcast_to(cnts[c].reshape(1, -1),
                               (P, cnts.shape[1] * cnts.shape[2])).copy()
        in_maps.append({
            "xT": xT, "xTs": xTs, "wcat": wcat, "vd": vdcat, "cnt": cntc,
            "l1w": np.asarray(inputs["lin1_w"], np.float32).astype(
                ml_dtypes.bfloat16),
            "l2w": np.asarray(inputs["lin2_w"], np.float32).astype(
                ml_dtypes.bfloat16),
            "l1b": l1b, "l2b": l2b, "bcat": bcat,
            "esrc": np.ascontiguousarray(esrc),
            "erel": np.ascontiguousarray(erel),
            "sT": np.ascontiguousarray(sTf[c]),
        })
    return in_maps, ch_lo, ch_hi, nidx


_CACHE = {}


def kernel(**inputs):
    cfg = Cfg()
    in_maps, ch_lo, ch_hi, nidx = preprocess(cfg, inputs)
    key = ("v6", ch_lo, ch_hi, nidx)
    if key not in _CACHE:
        _CACHE[key] = build_program(cfg, ch_lo, ch_hi, nidx)
    nc = _CACHE[key]
    res = run_bass_kernel_spmd(nc, in_maps, list(range(cfg.n_cores))).results
    takes = [min(cfg.npc, cfg.n - c * cfg.npc) for c in range(cfg.n_cores)]
    x_in = np.concatenate([res[c]["out_conv0"][:takes[c]]
                           for c in range(cfg.n_cores)])
    x_out = np.concatenate([res[c]["out_conv1"][:takes[c]]
                            for c in range(cfg.n_cores)])
    x_self = np.concatenate([res[c]["out_self"][:takes[c]]
                             for c in range(cfg.n_cores)])
    return (x_in, x_out, x_self)


# revision 18
# speedup vs baseline: 1.5320x; 1.0496x over previous
"""Distributed GAT (2x GATConv + MLP self-path) on 8 Trainium2 NeuronCores.

Strategy (dst-node graph parallelism, SPMD on 8 cores, v3):
  Host:
    - fold attention vectors into projection weights, cast x/weights to bf16,
      append self-loops, sort edges by dst, partition edges by 6272-node
      (128-aligned) dst blocks per core, group each core's dst nodes into
      128-node groups, order each group's edges [src<LO | src>=LO] with both
      sections padded to chunk counts ch_lo/ch_hi (global constants so the
      SPMD program is identical across cores); pad gather indices with -1.
    - xp table columns are C-MAJOR per conv ([c*32+h]) so the per-edge
      per-head weighting broadcasts along the INNER head axis (DVE 2x mode).
    - stream the per-chunk transposed one-hot dst-selector S_T (bf16) so the
      per-edge a_d term becomes a tiny on-device matmul instead of a gather.
    - per-(group,call) gather num_idxs statically trimmed to the max real
      count over the 8 cores (desc-gen on the Q7 costs ~10-18ns per padded
      slot, so padding is expensive).
  Device:
    - Phase S: MLP self path (bf16 matmuls) on own node block.
    - Phase A: a_d for own dst nodes from x_own @ v_d (kept in SBUF).
    - Phase T: projection table row(n) = [xp1_cm 512|xp2_cm 512|a_s1 32|
      a_s2 32|pad 64] bf16 (2304 B) written as two tables (node < LO /
      >= LO so int16 gather indices reach every row).
    - Phase E: per 128-dst-node group: one dma_gather per section for all
      edges (both convs share the row), scores exp(leakyrelu(a_s+a_d)-ln16)
      without max-subtraction (exact softmax identity, scale cancels),
      one-hot segment matmuls accumulate the weighted feature sum and the
      softmax denominator in one PSUM tile; normalisation, head-mean (inner
      reduce in c-major), bias, elu.
  Host: concatenate per-core dst blocks.
"""
import math
import numpy as np
import ml_dtypes

import concourse.bass as bass
import concourse.tile as tile
import concourse.mybir as mybir
import bass_rust
from concourse import bacc
from concourse.bass_utils import run_bass_kernel_spmd

AX_X = bass_rust.AxisListType.X
F32 = mybir.dt.float32
BF16 = mybir.dt.bfloat16
I16 = mybir.dt.int16
Act = mybir.ActivationFunctionType
Alu = mybir.AluOpType
P = 128
LN16 = float(np.log(16.0))


class Cfg:
    def __init__(self):
        self.n = 50000
        self.d_in = 256
        self.c = 16
        self.h = 32
        self.hd = self.h * self.c           # 512
        self.n_cores = 8
        self.npc = 6272                     # 128-aligned dst block per core
        self.groups = self.npc // P         # 49
        self.n_pad = self.npc * self.n_cores  # 50176
        self.nt = self.n_pad // P           # 392 node tiles
        self.kd = self.d_in // P            # 2
        self.lo = 32768
        self.ntl = self.lo // P             # 256 tiles in lo table
        self.n_hi = self.n_pad - self.lo    # 17408
        self.rowe = 1152                    # table row bf16 elements (w/ pad)
        self.used = 1088                    # meaningful row columns
        self.sub = 4                        # chunks per span
        self.gcall = 8                      # chunks per gather call (<=64
                                            # descriptors per SDMA engine)


def _elu(nc, pool, out_ap, in_ap, tag):
    shape = list(in_ap.shape)
    u = pool.tile(shape, F32, tag=tag + "_u")
    rl = pool.tile(shape, F32, tag=tag + "_r")
    nc.vector.tensor_scalar_min(out=u[:], in0=in_ap, scalar1=0.0)
    nc.scalar.activation(u[:], u[:], Act.Exp)
    nc.scalar.activation(rl[:], in_ap, Act.Relu)
    nc.vector.scalar_tensor_tensor(
        out=out_ap, in0=u[:], scalar=-1.0, in1=rl[:], op0=Alu.add, op1=Alu.add)


def build_program(cfg: Cfg, ch_lo: int, ch_hi: int, nidx, phases: str = "STE"):
    """nidx: per-group tuple of per-call static gather num_idxs (128-aligned,
    maxed over cores, <=1024 each; 0 = skip the call). Call windows are
    gcall-chunk slices of the lo then hi sections."""
    nc = bacc.Bacc("TRN2", target_bir_lowering=False, debug=False,
                   num_devices=cfg.n_cores)
    G, H, C, HD = cfg.groups, cfg.h, cfg.c, cfg.hd
    ch = ch_lo + ch_hi
    ROWE = cfg.rowe
    SUB = cfg.sub
    GC = cfg.gcall
    calls_lo = math.ceil(ch_lo / GC)
    calls_hi = math.ceil(ch_hi / GC)
    ncalls = calls_lo + calls_hi
    # (chunk base, table) per call window
    windows = [(ci * GC, 0) for ci in range(calls_lo)] + \
              [(ch_lo + ci * GC, 1) for ci in range(calls_hi)]

    t_xT = nc.dram_tensor("xT", [cfg.d_in, cfg.n_pad], BF16, kind="ExternalInput")
    t_xTs = nc.dram_tensor("xTs", [cfg.d_in, cfg.npc], BF16, kind="ExternalInput")
    t_wcat = nc.dram_tensor("wcat", [cfg.d_in, cfg.used], BF16, kind="ExternalInput")
    t_vd = nc.dram_tensor("vd", [cfg.d_in, 2 * H], BF16, kind="ExternalInput")
    t_l1w = nc.dram_tensor("l1w", [cfg.d_in, 4 * C], BF16, kind="ExternalInput")
    t_l2w = nc.dram_tensor("l2w", [4 * C, C], BF16, kind="ExternalInput")
    t_l1b = nc.dram_tensor("l1b", [4 * C, 1], F32, kind="ExternalInput")
    t_l2b = nc.dram_tensor("l2b", [P, C], F32, kind="ExternalInput")
    t_bcat = nc.dram_tensor("bcat", [P, 2 * C], F32, kind="ExternalInput")
    t_esrc = nc.dram_tensor("esrc", [P, G * ch * 8], I16, kind="ExternalInput")
    t_cnt = nc.dram_tensor("cnt", [P, G * ncalls], mybir.dt.int32,
                           kind="ExternalInput")
    t_erel = nc.dram_tensor("erel", [P, G * ch], BF16, kind="ExternalInput")
    t_sT = nc.dram_tensor("sT", [P, G * ch * P], BF16, kind="ExternalInput")

    t_oconv = [nc.dram_tensor(f"out_conv{i}", [cfg.npc, C], F32,
                              kind="ExternalOutput") for i in range(2)]
    t_oself = nc.dram_tensor("out_self", [cfg.npc, C], F32,
                             kind="ExternalOutput")

    t_tabL = nc.dram_tensor("tabL", [cfg.lo, ROWE], BF16)
    t_tabH = nc.dram_tensor("tabH", [cfg.n_hi, ROWE], BF16)

    with tile.TileContext(nc) as tc:
        import contextlib
        with contextlib.ExitStack() as ctx:
            cst = ctx.enter_context(tc.tile_pool(name="cst", bufs=1))
            wkp = ctx.enter_context(tc.tile_pool(name="wkp", bufs=2))
            stc = contextlib.ExitStack()
            cstT = stc.enter_context(tc.tile_pool(name="cstT", bufs=1))
            xtp = stc.enter_context(tc.tile_pool(name="xtp", bufs=3))
            tbp = stc.enter_context(tc.tile_pool(name="tbp", bufs=3))

            # ---- constants ----
            iota_i = cst.tile([P, P], mybir.dt.int32, tag="ioi")
            nc.gpsimd.iota(iota_i[:], pattern=[[1, P]], base=0, channel_multiplier=0)
            iota_b = cst.tile([P, P], BF16, tag="iob")
            nc.vector.tensor_copy(iota_b[:], iota_i[:])
            wcat = []
            for k in range(cfg.kd):
                w = cstT.tile([P, cfg.used], BF16, tag=f"wc{k}")
                nc.sync.dma_start(out=w[:], in_=t_wcat[k * P:(k + 1) * P, :])
                wcat.append(w)
            vd = []
            for k in range(cfg.kd):
                w = cstT.tile([P, 2 * H], BF16, tag=f"vd{k}")
                nc.sync.dma_start(out=w[:], in_=t_vd[k * P:(k + 1) * P, :])
                vd.append(w)
            l1w = []
            for k in range(cfg.kd):
                w = cstT.tile([P, 4 * C], BF16, tag=f"l1w{k}")
                nc.sync.dma_start(out=w[:], in_=t_l1w[k * P:(k + 1) * P, :])
                l1w.append(w)
            l2w = cstT.tile([4 * C, C], BF16, tag="l2w")
            nc.sync.dma_start(out=l2w[:], in_=t_l2w[:])
            l1b = cstT.tile([4 * C, 1], F32, tag="l1b")
            nc.sync.dma_start(out=l1b[:], in_=t_l1b[:])
            l2b = cstT.tile([P, C], F32, tag="l2b")
            nc.sync.dma_start(out=l2b[:], in_=t_l2b[:])
            bcat = cst.tile([P, 2, C], F32, tag="bcat")
            nc.sync.dma_start(out=bcat[:].rearrange("p u c -> p (u c)"),
                              in_=t_bcat[:])
            nl16 = cst.tile([P, 1], F32, tag="nl16")
            nc.gpsimd.memset(nl16[:], -LN16)
            erel = cst.tile([P, G * ch], BF16, tag="erel")
            nc.sync.dma_start(out=erel[:], in_=t_erel[:])
            cnt = cst.tile([P, G * ncalls], mybir.dt.int32, tag="cnt")
            nc.sync.dma_start(out=cnt[:], in_=t_cnt[:])
            adn = cst.tile([P, G, 2 * H], BF16, tag="adn")

            # ---- Phases S/A interleaved into T: one S block per 2 T
            # batches so S's vector work hides under T's matmul stream ----
            def emit_S_block(blk):
                    bs = min(512, cfg.npc - blk)
                    x1p = psS.tile([4 * C, 512], F32, tag="x1p")
                    xk = []
                    for k in range(cfg.kd):
                        xts = xtp.tile([P, 512], BF16, tag="xts")
                        nc.sync.dma_start(out=xts[:, :bs],
                                          in_=t_xTs[k * P:(k + 1) * P, blk:blk + bs])
                        xk.append(xts)
                        nc.tensor.matmul(out=x1p[:, :bs], lhsT=l1w[k][:],
                                         rhs=xts[:, :bs],
                                         start=(k == 0), stop=(k == cfg.kd - 1))
                    x1s = wkp.tile([4 * C, 512], BF16, tag="x1s")
                    nc.vector.tensor_add(out=x1s[:, :bs], in0=x1p[:, :bs],
                                         in1=l1b[:].to_broadcast([4 * C, bs]))
                    _elu(nc, wkp, x1s[:, :bs], x1s[:, :bs], "se")
                    for m in range(bs // P):
                        gi = (blk + m * P) // P
                        pst = psT.tile([P, C + 2 * H], F32, tag="pst")
                        nc.tensor.matmul(out=pst[:, 0:C],
                                         lhsT=x1s[:, m * P:(m + 1) * P],
                                         rhs=l2w[:], start=True, stop=True)
                        o2s = wkp.tile([P, C], F32, tag="o2s")
                        nc.vector.tensor_add(out=o2s[:], in0=pst[:, 0:C],
                                             in1=l2b[:])
                        _elu(nc, wkp, o2s[:], o2s[:], "so")
                        nc.sync.dma_start(
                            out=t_oself[blk + m * P:blk + (m + 1) * P, :],
                            in_=o2s[:])
                        # own-node a_d for this 128-node group
                        for k in range(cfg.kd):
                            nc.tensor.matmul(out=pst[:, C:C + 2 * H],
                                             lhsT=xk[k][:, m * P:(m + 1) * P],
                                             rhs=vd[k][:],
                                             start=(k == 0), stop=(k == cfg.kd - 1))
                        nc.vector.tensor_copy(out=adn[:, gi, :],
                                              in_=pst[:, C:C + 2 * H])

            # ---- Phase T: projection tables (+ interleaved S blocks) ----
            sblks = list(range(0, cfg.npc, 512)) if "S" in phases else []
            si = 0
            with tc.tile_pool(name="psS", bufs=1, space="PSUM") as psS, \
                 tc.tile_pool(name="psT", bufs=1, space="PSUM") as psT, \
                 tc.tile_pool(name="psA", bufs=2, space="PSUM") as psA:
                for bi, ts4 in enumerate(range(0, cfg.nt, 8)
                                         if "T" in phases else []):
                    if bi % 2 == 0 and si < len(sblks):
                        emit_S_block(sblks[si])
                        si += 1
                    mt = min(8, cfg.nt - ts4)
                    xk = []
                    for k in range(cfg.kd):
                        xt = xtp.tile([P, 1024], BF16, tag=f"xt{k}")
                        nc.sync.dma_start(
                            out=xt[:, :mt * P],
                            in_=t_xT[k * P:(k + 1) * P, ts4 * P:(ts4 + mt) * P])
                        xk.append(xt)
                    for m in range(mt):
                        ts = ts4 + m
                        pt = psA.tile([P, cfg.used], F32, tag="pt")
                        for k in range(cfg.kd):
                            for q0, q1 in ((0, 512), (512, 1024), (1024, 1088)):
                                nc.tensor.matmul(out=pt[:, q0:q1],
                                                 lhsT=xk[k][:, m * P:(m + 1) * P],
                                                 rhs=wcat[k][:, q0:q1],
                                                 start=(k == 0),
                                                 stop=(k == cfg.kd - 1))
                        # table row: [xp1_cm 512 | xp2_cm 512 | a_s1 32 |
                        #             a_s2 32 | pad 64]
                        stag = tbp.tile([P, ROWE], BF16, tag="stag")
                        nc.vector.tensor_copy(
                            out=stag[:, 0:256], in_=pt[:, 0:256])
                        nc.scalar.activation(
                            stag[:, 256:1088], pt[:, 256:1088], Act.Copy)
                        if ts < cfg.ntl:
                            rows = t_tabL[ts * P:(ts + 1) * P, :]
                        else:
                            rows = t_tabH[(ts - cfg.ntl) * P:(ts - cfg.ntl + 1) * P, :]
                        nc.sync.dma_start(out=rows, in_=stag[:])

            stc.close()

            # ---- Phase E: edge aggregation ----
            XB = 3
            with tc.tile_pool(name="gat", bufs=XB) as gat, \
                 tc.tile_pool(name="stp", bufs=2) as stp, \
                 tc.tile_pool(name="wsp", bufs=2) as wsp, \
                 tc.tile_pool(name="psE", bufs=2, space="PSUM") as psE, \
                 tc.tile_pool(name="psD", bufs=2, space="PSUM") as psD, \
                 nc.gpsimd.register("gcnt") as greg:
                npair = ch
                for gi in (range(G) if "E" in phases else []):
                    isrc = gat.tile([P, ch * 8], I16, tag="isrc")
                    nc.sync.dma_start(out=isrc[:],
                                      in_=t_esrc[:, gi * ch * 8:(gi + 1) * ch * 8])
                    sT = stp.tile([P, ch * P], BF16, tag="sT")
                    nc.sync.dma_start(out=sT[:],
                                      in_=t_sT[:, gi * ch * P:(gi + 1) * ch * P])
                    xpa = gat.tile([P, ch, ROWE], BF16, tag="xpa")
                    if gi < XB:
                        nc.gpsimd.memset(xpa[:], 0.0)
                    for ci, (c0, sec) in enumerate(windows):
                        nd = nidx[gi][ci]
                        if nd == 0:
                            continue
                        tab = t_tabL if sec == 0 else t_tabH
                        jw = (nd + P - 1) // P
                        nc.gpsimd.reg_load(
                            greg, cnt[0:1, gi * ncalls + ci:
                                      gi * ncalls + ci + 1])
                        nc.gpsimd.dma_gather(
                            out_ap=xpa[:, c0:c0 + jw, :], in_ap=tab[:],
                            idxs_ap=isrc[:, c0 * 8:c0 * 8 + nd // 16],
                            num_idxs=nd, num_idxs_reg=greg,
                            elem_size=ROWE)

                    out_ps = psE.tile([P, cfg.used], F32, tag="out_ps")
                    pair = 0
                    for c0 in range(0, ch, SUB):
                        jw = min(SUB, ch - c0)
                        # per-edge a_d via one-hot matmul from streamed S_T
                        ade = psD.tile([P, SUB, 2 * H], F32, tag="ade")
                        for j in range(jw):
                            nc.tensor.matmul(
                                out=ade[:, j, :],
                                lhsT=sT[:, (c0 + j) * P:(c0 + j + 1) * P],
                                rhs=adn[:, gi, :], start=True, stop=True)
                        # scores: z = a_s + a_d, leaky, exp(z - ln16)
                        scr = wsp.tile([P, SUB, 2 * H], BF16, tag="scr")
                        nc.vector.tensor_add(
                            out=scr[:, :jw, :].rearrange(
                                "p s (u t) -> p s u t", u=2),
                            in0=xpa[:, c0:c0 + jw, 1024:1088].rearrange(
                                "p s (u t) -> p s u t", u=2),
                            in1=ade[:, :jw, :].rearrange(
                                "p s (u t) -> p s u t", u=2))
                        nc.vector.scalar_tensor_tensor(
                            out=scr[:, :jw, :], in0=scr[:, :jw, :], scalar=0.2,
                            in1=scr[:, :jw, :], op0=Alu.mult, op1=Alu.max)
                        wsc = wsp.tile([P, SUB, cfg.used], BF16, tag="wsc")
                        nc.scalar.activation(
                            wsc[:, :jw, 1024:1088], scr[:, :jw, :], Act.Exp,
                            bias=nl16[:], scale=1.0)
                        # weighted features (c-major: bcast along inner h)
                        for i in range(2):
                            nc.vector.tensor_tensor(
                                out=wsc[:, :jw, i * HD:(i + 1) * HD].rearrange(
                                    "p s (c h) -> p s c h", c=C),
                                in0=xpa[:, c0:c0 + jw,
                                        i * HD:(i + 1) * HD].rearrange(
                                    "p s (c h) -> p s c h", c=C),
                                in1=wsc[:, :jw, 1024 + i * H:1024 + (i + 1) * H]
                                    .unsqueeze(2).to_broadcast([P, jw, C, H]),
                                op=Alu.mult)
                        # one-hot S for this span
                        S = wsp.tile([P, SUB, P], BF16, tag="S")
                        nc.vector.tensor_tensor(
                            out=S[:, :jw, :],
                            in0=erel[:, gi * ch + c0:gi * ch + c0 + jw]
                                .unsqueeze(2).to_broadcast([P, jw, P]),
                            in1=iota_b[:].unsqueeze(1).to_broadcast([P, jw, P]),
                            op=Alu.is_equal)
                        for jj in range(jw):
                            for q0, q1 in ((0, 512), (512, 1024), (1024, 1088)):
                                nc.tensor.matmul(
                                    out=out_ps[:, q0:q1],
                                    lhsT=S[:, jj, :],
                                    rhs=wsc[:, jj, q0:q1],
                                    start=(pair == 0), stop=(pair == npair - 1))
                            pair += 1
                    # normalize + head mean (inner) + bias + elu
                    rsb = wkp.tile([P, 2 * H], F32, tag="rsb")
                    nc.scalar.activation(rsb[:], out_ps[:, 1024:1088],
                                         Act.Copy, scale=float(H), bias=1e-12)
                    nc.vector.reciprocal(rsb[:], rsb[:])
                    tmp = wkp.tile([P, 2, C, H], F32, tag="tmp")
                    nc.vector.tensor_tensor(
                        out=tmp[:],
                        in0=out_ps[:, 0:1024].rearrange(
                            "p (u c h) -> p u c h", u=2, c=C),
                        in1=rsb[:].rearrange("p (u h) -> p u h", u=2)
                            .unsqueeze(2).to_broadcast([P, 2, C, H]),
                        op=Alu.mult)
                    om = wkp.tile([P, 2, C], F32, tag="om")
                    nc.vector.reduce_sum(out=om[:], in_=tmp[:], axis=AX_X)
                    nc.vector.tensor_add(out=om[:], in0=om[:], in1=bcat[:])
                    _elu(nc, wkp, om[:], om[:], "oe")
                    for i in range(2):
                        nc.sync.dma_start(
                            out=t_oconv[i][gi * P:(gi + 1) * P, :],
                            in_=om[:, i, :])
    nc.compile()
    return nc


def _wrap16(flat):
    """edge i -> [i%16, i//16], replicated to 128 partitions."""
    w = flat.reshape(-1, 16).T  # [16, len/16]
    return np.tile(w, (8, 1))


def preprocess(cfg: Cfg, inputs):
    n, H, C, G = cfg.n, cfg.h, cfg.c, cfg.groups
    x = np.asarray(inputs["x"], np.float32)
    ei = np.asarray(inputs["edge_index"])

    def fold(W, a_s, a_d):
        W = np.asarray(W, np.float32).reshape(cfg.d_in, H, C)
        v_s = (W * np.asarray(a_s, np.float32)[None]).sum(-1)
        v_d = (W * np.asarray(a_d, np.float32)[None]).sum(-1)
        # c-major projection columns: col (c*H + h) = W[:, h, c]
        W_cm = W.transpose(0, 2, 1).reshape(cfg.d_in, H * C)
        return W_cm, v_s, v_d

    W1, vs1, vd1 = fold(inputs["W1"], inputs["att_src1"], inputs["att_dst1"])
    W2, vs2, vd2 = fold(inputs["W2"], inputs["att_src2"], inputs["att_dst2"])
    # psum col layout: [xp1_cm 512 | xp2_cm 512 | a_s1 32 | a_s2 32]
    wcat = np.concatenate([W1, W2, vs1, vs2], 1).astype(ml_dtypes.bfloat16)
    vdcat = np.concatenate([vd1, vd2], 1).astype(ml_dtypes.bfloat16)

    xT = np.zeros((cfg.d_in, cfg.n_pad), ml_dtypes.bfloat16)
    xT[:, :n] = x.T.astype(ml_dtypes.bfloat16)

    loops = np.arange(n, dtype=np.int64)
    src = np.concatenate([ei[0], loops]).astype(np.int32)
    dst = np.concatenate([ei[1], loops]).astype(np.int32)
    order = np.argsort(dst, kind="stable")
    src_s = src[order]
    dst_s = dst[order]
    dev = dst_s // cfg.npc
    rel = dst_s - dev * cfg.npc
    grp = rel >> 7
    hi_e = (src_s >= cfg.lo).astype(np.int64)
    sidx = (dev * G + grp) * 2 + hi_e
    counts = np.bincount(sidx, minlength=cfg.n_cores * G * 2)
    c2 = counts.reshape(-1, 2)

    ch_lo = max(1, int(math.ceil(c2[:, 0].max() / P)))
    ch_hi = max(1, int(math.ceil(c2[:, 1].max() / P)))
    ch = ch_lo + ch_hi
    order2 = np.argsort(sidx, kind="stable")
    src2 = src_s[order2]
    sidx2 = sidx[order2]
    rel2 = (rel & 127)[order2]
    starts = np.zeros(len(counts), np.int64)
    starts[1:] = np.cumsum(counts)[:-1]
    pos_in_sec = np.arange(len(src2)) - starts[sidx2]
    dev2 = sidx2 // (G * 2)
    grp2 = (sidx2 // 2) % G
    hi2 = sidx2 & 1
    slot = np.where(hi2 == 0, pos_in_sec, ch_lo * P + pos_in_sec)
    gpos = grp2 * (ch * P) + slot

    GE = G * ch * P
    src16 = np.full((cfg.n_cores, GE), -1, np.int16)
    relf = np.full((cfg.n_cores, GE), -1.0, np.float32)
    sTf = np.zeros((cfg.n_cores, P, GE), ml_dtypes.bfloat16)
    src_adj = np.where(hi2 == 1, src2 - cfg.lo, src2).astype(np.int16)
    src16[dev2, gpos] = src_adj
    relf[dev2, gpos] = rel2.astype(np.float32)
    sTf[dev2, rel2, gpos] = 1.0

    # per-(core, group, section) real counts
    secn = np.zeros((cfg.n_cores, G, 2), np.int64)
    np.add.at(secn, (dev2, grp2, hi2), 1)
    # gather call windows: gcall-chunk slices of each section
    GC = cfg.gcall
    calls_lo = math.ceil(ch_lo / GC)
    calls_hi = math.ceil(ch_hi / GC)
    ncalls = calls_lo + calls_hi
    cnts = np.zeros((cfg.n_cores, G, ncalls), np.int32)
    nidx = []
    nmax = np.maximum(secn.max(axis=0), 1)          # [G, 2]
    for ci in range(ncalls):
        if ci < calls_lo:
            s, base = 0, ci * GC * P
            width = min(GC, ch_lo - ci * GC) * P
        else:
            s, base = 1, (ci - calls_lo) * GC * P
            width = min(GC, ch_hi - (ci - calls_lo) * GC) * P
        real = np.clip(secn[:, :, s] - base, 0, width)        # [cores, G]
        stat = np.clip(-(-(nmax[:, s] - base) // 16) * 16, 0, width)  # [G]
        cnts[:, :, ci] = np.where(real > 0, real, 1)
        nidx.append(stat)
        # sentinel index 0 for cores with an empty (but emitted) window
        need = (real == 0) & (stat[None, :] > 0)
        if need.any():
            cc, gg = np.nonzero(need)
            sec_off = 0 if s == 0 else ch_lo * P
            src16[cc, gg * (ch * P) + sec_off + base] = 0
    nidx = tuple(tuple(int(nidx[ci][g]) for ci in range(ncalls))
                 for g in range(G))

    l1b = np.asarray(inputs["lin1_b"], np.float32).reshape(4 * C, 1)
    l2b = np.broadcast_to(np.asarray(inputs["lin2_b"], np.float32), (P, C)).copy()
    bcat = np.broadcast_to(
        np.concatenate([np.asarray(inputs["b1"], np.float32),
                        np.asarray(inputs["b2"], np.float32)]), (P, 2 * C)).copy()

    in_maps = []
    for c in range(cfg.n_cores):
        xTs = np.zeros((cfg.d_in, cfg.npc), ml_dtypes.bfloat16)
        lo_n = min(cfg.npc, max(0, n - c * cfg.npc))
        xTs[:, :lo_n] = xT[:, c * cfg.npc:c * cfg.npc + lo_n]
        esrc = np.concatenate(
            [_wrap16(src16[c, gi * ch * P:(gi + 1) * ch * P])
             for gi in range(G)], axis=1)
        erel = np.concatenate(
            [relf[c, gi * ch * P:(gi + 1) * ch * P].reshape(ch, P).T
             for gi in range(G)], axis=1).astype(ml_dtypes.bfloat16)
        cntc = np.broadcast_to(cnts[c].reshape(1, -1),
                               (P, cnts.shape[1] * cnts.shape[2])).copy()
        in_maps.append({
            "xT": xT, "xTs": xTs, "wcat": wcat, "vd": vdcat, "cnt": cntc,
            "l1w": np.asarray(inputs["lin1_w"], np.float32).astype(
                ml_dtypes.bfloat16),
            "l2w": np.asarray(inputs["lin2_w"], np.float32).astype(
                ml_dtypes.bfloat16),
            "l1b": l1b, "l2b": l2b, "bcat": bcat,
            "esrc": np.ascontiguousarray(esrc),
            "erel": np.ascontiguousarray(erel),
            "sT": np.ascontiguousarray(sTf[c]),
        })
    return in_maps, ch_lo, ch_hi, nidx


_CACHE = {}


def kernel(**inputs):
    cfg = Cfg()
    in_maps, ch_lo, ch_hi, nidx = preprocess(cfg, inputs)
    key = ("v7", ch_lo, ch_hi, nidx)
    if key not in _CACHE:
        _CACHE[key] = build_program(cfg, ch_lo, ch_hi, nidx)
    nc = _CACHE[key]
    res = run_bass_kernel_spmd(nc, in_maps, list(range(cfg.n_cores))).results
    takes = [min(cfg.npc, cfg.n - c * cfg.npc) for c in range(cfg.n_cores)]
    x_in = np.concatenate([res[c]["out_conv0"][:takes[c]]
                           for c in range(cfg.n_cores)])
    x_out = np.concatenate([res[c]["out_conv1"][:takes[c]]
                            for c in range(cfg.n_cores)])
    x_self = np.concatenate([res[c]["out_self"][:takes[c]]
                             for c in range(cfg.n_cores)])
    return (x_in, x_out, x_self)
